# revision 26
# baseline (speedup 1.0000x reference)
"""MemN2N Bass kernel builder (per-core program, SPMD over 8 cores).

Per-core work (core c):
  - 8 local batches (B_LOC). story_pad [TOT_SLOTS, SENT] int32 staged so that
    slot(16b+q, j) = batch b, sentence 13q+j  (S_PAD sents/batch, SPP per part).
    Pad tokens point at table row V (a zero row appended host-side).
  - The 4 tables are concatenated host-side into embcat [V+1, 4E] bf16 so one
    gathered row covers all tables; one indirect DMA per (j) gathers all
    128x50 rows (amortizes the ~1us SWDGE per-instruction fixed cost).
  - Token-sum via unit-stride halving adds on DVE -> G [128, SPP, 4E] bf16.
  - 3 attention hops (PE transposes for G^T, scores matmul, softmax on ACT,
    DRAM-bounce repack, block-diag combine matmul) -> u3.
  - logits = u3 @ emb3.T via emb3T bf16 [E, VPAD] staged pre-transposed;
    softmax over vocab computed on-device; output [B_LOC, V] f32.
"""
import sys

sys.path.insert(0, "/opt/trn_rl_repo")

from contextlib import ExitStack

import numpy as np

import concourse.bass as bass
import concourse.mybir as mybir
import concourse.tile as tile
from concourse.masks import make_identity

F32 = mybir.dt.float32
BF16 = mybir.dt.bfloat16
F8 = mybir.dt.float8e4
I32 = mybir.dt.int32
GATHER_FP8 = True  # embcat stored e4m3: halves gather bytes; ~2% row error
AX = mybir.AxisListType
ALU = mybir.AluOpType
ACTF = mybir.ActivationFunctionType

P = 128
E = 128


class Cfg:
    def __init__(self, B_LOC=8, S=200, SENT=50, V=100000, K_HOP=3, CHUNK_VT=32):
        self.B_LOC = B_LOC
        self.S = S
        self.SENT = SENT
        self.V = V
        self.K_HOP = K_HOP
        self.NT = K_HOP + 1
        self.EC = self.NT * E  # concat row width
        self.PPB = P // B_LOC  # partitions per batch
        self.QI = -(-SENT // self.PPB)  # question tokens per partition
        # S_PAD: sentences per batch padded so B_LOC*S_PAD = 128*SPP
        self.SPP = -(-(B_LOC * S) // P)  # ceil
        self.S_PAD = self.PPB * self.SPP
        assert self.S_PAD >= S
        self.TOT_SLOTS = P * self.SPP
        # vocab padding for 128-row tiles
        self.NVT = -(-V // P)  # number of V tiles
        self.VPAD = self.NVT * P
        self.LAST_VT_ROWS = V - (self.NVT - 1) * P  # valid rows in last V tile
        # final-phase chunking: CHUNK_VT V-tiles of logits per psum/exp chunk
        self.CHUNK_VT = CHUNK_VT
        self.NCH = -(-self.NVT // CHUNK_VT)


def build_kernel(cfg: Cfg, nc: bass.Bass, dbg: bool = False):
    c = cfg
    # ---- I/O ----
    story = nc.declare_dram_parameter("story_pad", [c.TOT_SLOTS, c.SENT], I32, isOutput=False)
    # question tokens packed [128, QI]: partition PPB*b + r holds tokens
    # QI*r .. QI*r+QI-1 of batch b (pads -> row V)
    quest = nc.declare_dram_parameter("question", [P, c.QI], I32, isOutput=False)
    embcat = nc.declare_dram_parameter(
        "embcat", [c.V + 1, c.EC], F8 if GATHER_FP8 else BF16, isOutput=False
    )
    emb3T = nc.declare_dram_parameter("emb3T", [E, c.VPAD], BF16, isOutput=False)
    bmask_bf = nc.declare_dram_parameter("bmask_bf", [P, c.B_LOC], BF16, isOutput=False)
    bmask2 = nc.declare_dram_parameter("bmask2", [P, c.B_LOC], F32, isOutput=False)
    out = nc.declare_dram_parameter("out", [c.B_LOC, c.V], F32, isOutput=True)

    dbgout = None
    if dbg:
        dbgout = {
            "dG": nc.declare_dram_parameter("dG", [P, c.SPP * c.EC], F32, isOutput=True),
            "duT": nc.declare_dram_parameter("duT", [P, c.B_LOC], F32, isOutput=True),
        }
    with tile.TileContext(nc) as tc:
        _body(c, nc, tc, story, quest, embcat, emb3T, bmask_bf, bmask2, out, dbgout)
    return nc


def _tree_sum(nc, scratch, src, dst, np_, ec):
    """dst[:, 0, :] = sum over the 50-token axis of src [np_, 50, ec].

    Unit-stride halving adds into a shared bf16 scratch tile [P, 48, ec].
    First level split into two ops so no single DVE op blocks the gather
    stream for long. src may be fp8; scratch/dst are bf16.
    """
    s_ = lambda a, b: scratch[:np_, a:b, :]
    # a1[i] = src[i] + src[13+i], i<13 -> cols 0:13; a2 covers 26..49 -> 13:25
    nc.vector.tensor_add(out=s_(0, 13), in0=src[:np_, 0:13, :], in1=src[:np_, 13:26, :])
    nc.vector.tensor_add(out=s_(13, 25), in0=src[:np_, 26:38, :], in1=src[:np_, 38:50, :])
    nc.vector.tensor_add(out=s_(25, 31), in0=s_(0, 6), in1=s_(6, 12))
    nc.vector.tensor_add(out=s_(31, 37), in0=s_(13, 19), in1=s_(19, 25))
    nc.vector.tensor_add(out=s_(37, 40), in0=s_(25, 28), in1=s_(28, 31))
    nc.vector.tensor_add(out=s_(40, 43), in0=s_(31, 34), in1=s_(34, 37))
    nc.vector.tensor_add(out=s_(43, 46), in0=s_(37, 40), in1=s_(40, 43))
    nc.vector.tensor_add(out=s_(46, 47), in0=s_(43, 44), in1=s_(44, 45))
    nc.vector.tensor_add(out=s_(47, 48), in0=s_(46, 47), in1=s_(45, 46))
    # + leftover a1[12] (holds src[12] + src[25])
    nc.vector.tensor_add(out=dst, in0=s_(47, 48), in1=s_(12, 13))


def _body(c: Cfg, nc, tc, story, quest, embcat, emb3T, bmask_bf, bmask2, out, dbgout=None):
    with ExitStack() as es:
        # ---------- persistent pools ----------
        cpool = es.enter_context(tc.tile_pool(name="const", bufs=1))
        gpool = es.enter_context(tc.tile_pool(name="G", bufs=1))
        upool = es.enter_context(tc.tile_pool(name="u", bufs=1))

        identity = cpool.tile([P, P], F32)
        make_identity(nc, identity[:])
        identity_bf = cpool.tile([P, P], BF16)
        nc.vector.tensor_copy(out=identity_bf[:], in_=identity[:])

        # story indices resident in SBUF: [128, SPP*SENT]
        idx_t = cpool.tile([P, c.SPP * c.SENT], I32)
        nc.sync.dma_start(
            out=idx_t[:],
            in_=story[:].rearrange("(p j) t -> p (j t)", p=P),
        )
        qidx_t = cpool.tile([P, c.QI], I32)
        nc.sync.dma_start(out=qidx_t[:], in_=quest[:])
        bmask_t = cpool.tile([P, c.B_LOC], BF16)
        nc.sync.dma_start(out=bmask_t[:], in_=bmask_bf[:])
        bmask2_t = cpool.tile([P, c.B_LOC], F32)
        nc.sync.dma_start(out=bmask2_t[:], in_=bmask2[:])

        # G concat table [128, SPP, 4E] bf16; G_t = G[:, :, t*E:(t+1)*E]
        G = gpool.tile([P, c.SPP, c.EC], BF16, tag="G", name="G")
        # G^T for m-tables [E=128, TOT_SLOTS]
        GT = [gpool.tile([P, c.TOT_SLOTS], BF16, tag=f"GT{t}", name=f"GT{t}") for t in range(c.K_HOP)]

        # ---------- gather + segment-reduce ----------
        GDT = F8 if GATHER_FP8 else BF16
        with (
            tc.tile_pool(name="gather", bufs=3) as gbpool,
            tc.tile_pool(name="red", bufs=1) as rpool,
        ):
            scratch = rpool.tile([P, 48, c.EC], BF16, tag="scr")
            for j in range(c.SPP):
                gbuf = gbpool.tile([P, c.SENT, c.EC], GDT, tag="gbuf")
                # NB: HW SWDGE reads ONE offset per partition per instruction
                # (extra offset-AP columns are ignored and the free dim is
                # filled with consecutive rows — probed); so one instruction
                # per token column, 2D out slice, exactly like the baseline.
                for s in range(c.SENT):
                    nc.gpsimd.indirect_dma_start(
                        out=gbuf[:, s, :],
                        out_offset=None,
                        in_=embcat[:],
                        in_offset=bass.IndirectOffsetOnAxis(
                            ap=idx_t[:, j * c.SENT + s : j * c.SENT + s + 1],
                            axis=0,
                        ),
                    )
                _tree_sum(nc, scratch, gbuf, G[:, j : j + 1, :], P, c.EC)
            # question gather: [128, QI] indices -> per-partition partial sums
            # q4 [128, EC]; per-batch token sum finished on PE via bmask
            qbuf = gbpool.tile([P, c.SENT, c.EC], GDT, tag="gbuf")
            for s in range(c.QI):
                nc.gpsimd.indirect_dma_start(
                    out=qbuf[:, s, :],
                    out_offset=None,
                    in_=embcat[:],
                    in_offset=bass.IndirectOffsetOnAxis(
                        ap=qidx_t[:, s : s + 1], axis=0
                    ),
                )
            q4 = upool.tile([P, c.EC], BF16, tag="q4")
            assert c.QI == 4
            nc.vector.tensor_add(
                out=scratch[:, 0:1, :], in0=qbuf[:, 0:1, :], in1=qbuf[:, 1:2, :]
            )
            nc.vector.tensor_add(
                out=scratch[:, 1:2, :], in0=qbuf[:, 2:3, :], in1=qbuf[:, 3:4, :]
            )
            nc.vector.tensor_add(
                out=q4[:].unsqueeze(1), in0=scratch[:, 0:1, :], in1=scratch[:, 1:2, :]
            )

        # ---------- transposes: GT_t from G_t; uT0 from u0 ----------
        with tc.tile_pool(name="tp", bufs=4, space="PSUM") as tppool:
            for t in range(c.K_HOP):
                for j in range(c.SPP):
                    tp = tppool.tile([P, P], F32, tag="tp")
                    nc.tensor.matmul(
                        out=tp[:], lhsT=G[:, j, t * E : (t + 1) * E], rhs=identity_bf[:],
                        start=True, stop=True,
                    )
                    # psum col p <-> slot 13p+j: write GT[:, j::SPP]
                    nc.vector.tensor_copy(
                        out=GT[t][:].rearrange("e (p j) -> e p j", j=c.SPP)[:, :, j],
                        in_=tp[:],
                    )
            # uT[e, b] = sum_p q4[p, e] * bmask[p, b]  (finishes the question
            # token sum across the PPB partitions of each batch, pre-transposed)
            uT = upool.tile([P, c.B_LOC], F32, tag="uT")
            uT_bf = upool.tile([P, c.B_LOC], BF16, tag="uT_bf")
            tpu = tppool.tile([P, c.B_LOC], F32, tag="tpu")
            nc.tensor.matmul(
                out=tpu[:], lhsT=q4[:, :E], rhs=bmask_t[:],
                start=True, stop=True,
            )
            nc.vector.tensor_copy(out=uT[:], in_=tpu[:])
            nc.vector.tensor_copy(out=uT_bf[:], in_=tpu[:])

        if dbgout is not None:
            # gpsimd dma casts bf16 -> f32 on the way out
            nc.gpsimd.dma_start(
                out=dbgout["dG"][:],
                in_=G[:].rearrange("p a b -> p (a b)"),
            )
            nc.sync.dma_start(out=dbgout["duT"][:], in_=uT[:])

        # ---------- K_HOP attention hops ----------
        with (
            tc.tile_pool(name="hop", bufs=2) as hpool,
            tc.tile_pool(name="hop_ps", bufs=1, space="PSUM") as hpspool,
            tc.tile_pool(name="hop_ps2", bufs=2, space="PSUM") as hpspool2,
        ):
            for h in range(c.K_HOP):
                # scores [B_LOC, TOT_SLOTS] = uT.T @ GT[h]
                sc_ps = hpspool.tile([c.B_LOC, c.TOT_SLOTS], F32, tag="sc")
                for c0 in range(0, c.TOT_SLOTS, 512):
                    c1 = min(c0 + 512, c.TOT_SLOTS)
                    nc.tensor.matmul(
                        out=sc_ps[:, c0:c1],
                        lhsT=uT_bf[:],
                        rhs=GT[h][:, c0:c1],
                        start=True,
                        stop=True,
                    )
                # move scores to SBUF, bounce via DRAM with a diagonal AP to
                # get per-batch aligned scores scal[b, s] = scores[b, S_PAD*b + s]
                sc_sb = hpool.tile([c.B_LOC, c.TOT_SLOTS], F32, tag="sc_sb")
                nc.vector.tensor_copy(out=sc_sb[:], in_=sc_ps[:])
                scd = nc.dram_tensor(f"scd{h}", [c.B_LOC * c.TOT_SLOTS], F32)
                nc.sync.dma_start(
                    out=scd[:].rearrange("(b t) -> b t", t=c.TOT_SLOTS), in_=sc_sb[:]
                )
                diag = bass.AP(
                    tensor=scd[:].tensor,
                    offset=0,
                    ap=[[c.TOT_SLOTS + c.S_PAD, c.B_LOC], [1, c.S_PAD]],
                )
                scal = hpool.tile([c.B_LOC, c.S_PAD], F32, tag="scal")
                nc.sync.dma_start(out=scal[:], in_=diag)
                # masked softmax over the S real sentences
                probs = hpool.tile([c.B_LOC, c.S_PAD], F32, tag="probs")
                nc.vector.memset(probs[:], 0.0)
                negmax = hpool.tile([c.B_LOC, 1], F32, tag="negmax")
                nc.vector.tensor_reduce(
                    out=negmax[:], in_=scal[:, : c.S], axis=AX.X, op=ALU.max, negate=True
                )
                denom = hpool.tile([c.B_LOC, 1], F32, tag="denom")
                nc.scalar.activation(
                    out=probs[:, : c.S],
                    in_=scal[:, : c.S],
                    func=ACTF.Exp,
                    bias=negmax[:],
                    scale=1.0,
                    accum_out=denom[:],
                )
                rec = hpool.tile([c.B_LOC, 1], F32, tag="rec")
                nc.vector.reciprocal(out=rec[:], in_=denom[:])
                nc.vector.tensor_scalar_mul(probs[:, : c.S], probs[:, : c.S], rec[:])
                # repack probs [B_LOC, S_PAD] -> slot layout [128, SPP] via DRAM bounce
                pd = nc.dram_tensor(f"pd{h}", [c.TOT_SLOTS], F32)
                nc.sync.dma_start(
                    out=pd[:].rearrange("(b s) -> b s", s=c.S_PAD), in_=probs[:]
                )
                pslot = hpool.tile([P, c.SPP], F32, tag="pslot")
                nc.sync.dma_start(
                    out=pslot[:], in_=pd[:].rearrange("(p j) -> p j", j=c.SPP)
                )
                pslot_bf = hpool.tile([P, c.SPP], BF16, tag="pslot_bf")
                nc.vector.tensor_copy(out=pslot_bf[:], in_=pslot[:])
                # block-diagonal probs [128, SPP, B_LOC] = pslot (bcast) * bmask (bcast)
                bd = hpool.tile([P, c.SPP, c.B_LOC], BF16, tag="bd")
                nc.vector.tensor_tensor(
                    out=bd[:],
                    in0=pslot_bf[:].unsqueeze(-1).to_broadcast([P, c.SPP, c.B_LOC]),
                    in1=bmask_t[:].unsqueeze(1).to_broadcast([P, c.SPP, c.B_LOC]),
                    op=ALU.mult,
                )
                # combine: uT_new = sum_j G[h+1][:,j,:].T @ bd[:,j,:]  (+ uT)
                uc_ps = hpspool2.tile([P, c.B_LOC], F32, tag="uc")
                for j in range(c.SPP):
                    nc.tensor.matmul(
                        out=uc_ps[:],
                        lhsT=G[:, j, (h + 1) * E : (h + 2) * E],
                        rhs=bd[:, j, :],
                        start=(j == 0),
                        stop=(j == c.SPP - 1),
                    )
                uT_new = upool.tile([P, c.B_LOC], F32, tag=f"uT{h + 1}")
                nc.vector.tensor_add(out=uT_new[:], in0=uc_ps[:], in1=uT[:])
                uT = uT_new
                uT_bf_new = upool.tile([P, c.B_LOC], BF16, tag=f"uT_bf{h + 1}")
                nc.vector.tensor_copy(out=uT_bf_new[:], in_=uT[:])
                uT_bf = uT_bf_new

        # ---------- final phase: logits + vocab softmax ----------
        with (
            tc.tile_pool(name="fin", bufs=1) as fpool,
            tc.tile_pool(name="emb3c", bufs=2) as epool,
            tc.tile_pool(name="fin_ps", bufs=2, space="PSUM") as fps,
            tc.tile_pool(name="den_ps", bufs=1, space="PSUM") as dps,
            tc.tile_pool(name="out_ps", bufs=4, space="PSUM") as ops,
            tc.tile_pool(name="outsb", bufs=4) as osb,
        ):
            ones = fpool.tile([P, P], F32)
            nc.vector.memset(ones[:], 1.0)
            ones_part = fpool.tile([P, P], F32)
            nc.vector.memset(ones_part[:], 0.0)
            nc.vector.memset(ones_part[: c.LAST_VT_ROWS, :], 1.0)

            exp_buf = fpool.tile([P, c.NVT * c.B_LOC], F32)
            CW = c.CHUNK_VT * c.B_LOC  # psum/exp cols per chunk
            den_ps = dps.tile([P, CW], F32)
            for ch in range(c.NCH):
                vt0 = ch * c.CHUNK_VT
                nvt = min(c.CHUNK_VT, c.NVT - vt0)
                echunk = epool.tile([P, c.CHUNK_VT * P], BF16, tag="echunk")
                nc.sync.dma_start(
                    out=echunk[:, : nvt * P],
                    in_=emb3T[:, vt0 * P : (vt0 + nvt) * P],
                )
                lg_ps = fps.tile([P, CW], F32, tag="lg")
                for m in range(nvt):
                    nc.tensor.matmul(
                        out=lg_ps[:, m * c.B_LOC : (m + 1) * c.B_LOC],
                        lhsT=echunk[:, m * P : (m + 1) * P],
                        rhs=uT_bf[:],
                        start=True,
                        stop=True,
                    )
                ecols = nvt * c.B_LOC
                nc.scalar.activation(
                    out=exp_buf[:, vt0 * c.B_LOC : vt0 * c.B_LOC + ecols],
                    in_=lg_ps[:, :ecols],
                    func=ACTF.Exp,
                )
                # denominator partials: ones^T @ exp_chunk, accumulated in psum
                exp_ch = exp_buf[:, vt0 * c.B_LOC : vt0 * c.B_LOC + ecols]
                last_has_partial = vt0 + nvt == c.NVT and c.LAST_VT_ROWS < P
                full_cols = ecols - (c.B_LOC if last_has_partial else 0)
                if full_cols > 0:
                    nc.tensor.matmul(
                        out=den_ps[:, :full_cols],
                        lhsT=ones[:],
                        rhs=exp_ch[:, :full_cols],
                        start=(ch == 0),
                        stop=False,
                        skip_group_check=True,
                    )
                if last_has_partial:
                    nc.tensor.matmul(
                        out=den_ps[:, full_cols:ecols],
                        lhsT=ones_part[:],
                        rhs=exp_ch[:, full_cols:ecols],
                        start=False,
                        stop=True,
                        skip_group_check=True,
                    )
            # denominators [1, B_LOC] then reciprocal replicated to [128,1]
            den8 = fpool.tile([P, c.B_LOC], F32)
            nc.vector.tensor_reduce(
                out=den8[:].unsqueeze(-1),
                in_=den_ps[:].rearrange("o (m b) -> o b m", b=c.B_LOC),
                axis=AX.X,
                op=ALU.add,
            )
            rec8 = fpool.tile([P, c.B_LOC], F32)
            nc.vector.reciprocal(out=rec8[:], in_=den8[:])
            # rec_rep[p] = rec8[p % B_LOC] via mask multiply + free reduce
            rec_full = fpool.tile([P, c.B_LOC], F32)
            nc.vector.tensor_tensor(
                out=rec_full[:],
                in0=bmask2_t[:],
                in1=rec8[:],
                op=ALU.mult,
            )
            rec_rep = fpool.tile([P, 1], F32)
            nc.vector.tensor_reduce(
                out=rec_rep[:], in_=rec_full[:], axis=AX.X, op=ALU.add
            )

            # transpose 16-V-tile groups, scale by recip, DMA out
            GRP = P // c.B_LOC  # V tiles per transpose group
            ngrp = -(-c.NVT // GRP)
            n_full_vt = c.V // P  # V tiles fully inside the real vocab
            out3 = out[:, : n_full_vt * P].rearrange("b (t col) -> t b col", col=P)
            for g in range(ngrp):
                t0 = g * GRP
                nt = min(GRP, c.NVT - t0)
                cols = nt * c.B_LOC
                tps = ops.tile([P, P], F32, tag="otp")
                nc.tensor.matmul(
                    out=tps[:cols, :],
                    lhsT=exp_buf[:, t0 * c.B_LOC : t0 * c.B_LOC + cols],
                    rhs=identity[:],
                    start=True,
                    stop=True,
                )
                sb = osb.tile([P, P], F32, tag="osb")
                nc.vector.tensor_scalar_mul(sb[:cols, :], tps[:cols, :], rec_rep[:cols, :])
                # rows b + B_LOC*t', t' = local V-tile; tail V tile may be partial
                full_t = min(nt, n_full_vt - t0)
                if full_t > 0:
                    nc.sync.dma_start(
                        out=out3[t0 : t0 + full_t],
                        in_=sb[: full_t * c.B_LOC, :],
                    )
                if full_t < nt:  # partial last V tile
                    nc.sync.dma_start(
                        out=out[:, n_full_vt * P : c.V],
                        in_=sb[full_t * c.B_LOC : cols, : c.V - n_full_vt * P],
                    )


# ---------------- host-side pack/unpack ----------------
def ref_numpy(story, question, emb_A):
    """Full-batch numpy reference (mirrors reference.py)."""
    K_HOP = emb_A.shape[0] - 1
    u = emb_A[0][question].sum(axis=1)
    for i in range(K_HOP):
        m = emb_A[i][story].sum(axis=2)
        cc = emb_A[i + 1][story].sum(axis=2)
        logits_att = np.einsum("bse,be->bs", m, u)
        pa = np.exp(logits_att - logits_att.max(-1, keepdims=True))
        probs = pa / pa.sum(-1, keepdims=True)
        u = np.einsum("bse,bs->be", cc, probs) + u
    logits = u @ emb_A[-1].T
    z = np.exp(logits - logits.max(-1, keepdims=True))
    return (z / z.sum(-1, keepdims=True)).astype(np.float32)


N_CORES = 8
_CACHE = {}


def _get_nc(cfg):
    key = "nc"
    if key not in _CACHE:
        import concourse.bacc as bacc

        nc = bacc.Bacc(target_bir_lowering=False)
        build_kernel(cfg, nc)
        nc.finalize()
        _CACHE[key] = nc
    return _CACHE[key]


def _pack_shared(cfg, emb_A):
    key = "shared"
    if key not in _CACHE or _CACHE[key][0] is not emb_A:
        c = cfg
        import ml_dtypes

        embs = {}
        # concat tables [V, NT*E] + zero pad row
        ecat = np.zeros((c.V + 1, c.EC), np.float32)
        ecat[: c.V] = np.concatenate([emb_A[t] for t in range(c.NT)], axis=1)
        gdt = ml_dtypes.float8_e4m3 if GATHER_FP8 else ml_dtypes.bfloat16
        embs["embcat"] = ecat.astype(gdt)
        e3T = np.zeros((E, c.VPAD), np.float32)
        e3T[:, : c.V] = emb_A[c.NT - 1].T
        embs["emb3T"] = e3T.astype(ml_dtypes.bfloat16)
        bm = np.zeros((P, c.B_LOC), np.float32)
        for b in range(c.B_LOC):
            bm[b * c.PPB : (b + 1) * c.PPB, b] = 1.0
        embs["bmask_bf"] = bm.astype(ml_dtypes.bfloat16)
        bm2 = np.zeros((P, c.B_LOC), np.float32)
        for p in range(P):
            bm2[p, p % c.B_LOC] = 1.0
        embs["bmask2"] = bm2
        _CACHE[key] = (emb_A, embs)
    return _CACHE[key][1]


def _pack_story(cfg, story_c):
    c = cfg
    story_pad = np.full((c.B_LOC, c.S_PAD, c.SENT), c.V, np.int32)
    story_pad[:, : c.S, :] = story_c
    return np.ascontiguousarray(story_pad.reshape(c.TOT_SLOTS, c.SENT))


def _pack_question(cfg, quest_c):
    """[B_LOC, SENT] -> [128, QI]: partition PPB*b + r holds tokens
    QI*r .. QI*r+QI-1 of batch b (pads -> V)."""
    c = cfg
    qp = np.full((c.B_LOC, c.PPB * c.QI), c.V, np.int32)
    qp[:, : c.SENT] = quest_c
    return np.ascontiguousarray(qp.reshape(P, c.QI))


def kernel(story, question, emb_A, _trace=False, _trace_kwargs=None):
    from concourse import bass_utils

    story = np.asarray(story)
    question = np.asarray(question)
    emb_A = np.asarray(emb_A)

    cfg = Cfg(
        B_LOC=story.shape[0] // N_CORES,
        S=story.shape[1],
        SENT=story.shape[2],
        V=emb_A.shape[1],
        K_HOP=emb_A.shape[0] - 1,
    )
    nc = _get_nc(cfg)
    shared = _pack_shared(cfg, emb_A)
    in_maps = []
    for ci in range(N_CORES):
        sl = slice(ci * cfg.B_LOC, (ci + 1) * cfg.B_LOC)
        in_maps.append(
            {
                "story_pad": _pack_story(cfg, story[sl]),
                "question": _pack_question(cfg, np.asarray(question[sl]).astype(np.int32)),
                **shared,
            }
        )
    kwargs = {}
    if _trace:
        kwargs = dict(trace=True, trace_kwargs=_trace_kwargs or {})
    res = bass_utils.run_bass_kernel_spmd(
        nc, in_maps, core_ids=list(range(N_CORES)), **kwargs
    )
    out = np.concatenate([r["out"] for r in res.results], axis=0)
    if _trace:
        return out, res
    return out


# revision 28
# speedup vs baseline: 1.2405x; 1.2405x over previous
"""MemN2N Bass kernel builder (per-core program, SPMD over 8 cores).

Per-core work (core c):
  - 8 local batches (B_LOC). story_pad [TOT_SLOTS, SENT] int32 staged so that
    slot(16b+q, j) = batch b, sentence 13q+j  (S_PAD sents/batch, SPP per part).
    Pad tokens point at table row V (a zero row appended host-side).
  - The 4 tables are concatenated host-side into embcat [V+1, 4E] bf16 so one
    gathered row covers all tables; one indirect DMA per (j) gathers all
    128x50 rows (amortizes the ~1us SWDGE per-instruction fixed cost).
  - Token-sum via unit-stride halving adds on DVE -> G [128, SPP, 4E] bf16.
  - 3 attention hops (PE transposes for G^T, scores matmul, softmax on ACT,
    DRAM-bounce repack, block-diag combine matmul) -> u3.
  - logits = u3 @ emb3.T via emb3T bf16 [E, VPAD] staged pre-transposed;
    softmax over vocab computed on-device; output [B_LOC, V] f32.
"""
import sys

sys.path.insert(0, "/opt/trn_rl_repo")

from contextlib import ExitStack

import numpy as np

import concourse.bass as bass
import concourse.mybir as mybir
import concourse.tile as tile
from concourse.masks import make_identity

F32 = mybir.dt.float32
BF16 = mybir.dt.bfloat16
F8 = mybir.dt.float8e4
I32 = mybir.dt.int32
GATHER_FP8 = False  # e4m3 embcat measured slower (DVE fp8 adds) and noisier
AX = mybir.AxisListType
ALU = mybir.AluOpType
ACTF = mybir.ActivationFunctionType

P = 128
E = 128


class Cfg:
    def __init__(self, B_LOC=8, S=200, SENT=50, V=100000, K_HOP=3, CHUNK_VT=32):
        self.B_LOC = B_LOC
        self.S = S
        self.SENT = SENT
        self.V = V
        self.K_HOP = K_HOP
        self.NT = K_HOP + 1
        self.EC = self.NT * E  # concat row width
        self.PPB = P // B_LOC  # partitions per batch
        self.QI = -(-SENT // self.PPB)  # question tokens per partition
        # S_PAD: sentences per batch padded so B_LOC*S_PAD = 128*SPP
        self.SPP = -(-(B_LOC * S) // P)  # ceil
        self.S_PAD = self.PPB * self.SPP
        assert self.S_PAD >= S
        self.TOT_SLOTS = P * self.SPP
        # vocab padding for 128-row tiles
        self.NVT = -(-V // P)  # number of V tiles
        self.VPAD = self.NVT * P
        self.LAST_VT_ROWS = V - (self.NVT - 1) * P  # valid rows in last V tile
        # final-phase chunking: CHUNK_VT V-tiles of logits per psum/exp chunk
        self.CHUNK_VT = CHUNK_VT
        self.NCH = -(-self.NVT // CHUNK_VT)


def build_kernel(cfg: Cfg, nc: bass.Bass, dbg: bool = False):
    c = cfg
    # ---- I/O ----
    story = nc.declare_dram_parameter("story_pad", [c.TOT_SLOTS, c.SENT], I32, isOutput=False)
    # question tokens packed [128, QI]: partition PPB*b + r holds tokens
    # QI*r .. QI*r+QI-1 of batch b (pads -> row V)
    quest = nc.declare_dram_parameter("question", [P, c.QI], I32, isOutput=False)
    embcat = nc.declare_dram_parameter(
        "embcat", [c.V + 1, c.EC], F8 if GATHER_FP8 else BF16, isOutput=False
    )
    emb3T = nc.declare_dram_parameter("emb3T", [E, c.VPAD], BF16, isOutput=False)
    bmask_bf = nc.declare_dram_parameter("bmask_bf", [P, c.B_LOC], BF16, isOutput=False)
    bmask2 = nc.declare_dram_parameter("bmask2", [P, c.B_LOC], F32, isOutput=False)
    out = nc.declare_dram_parameter("out", [c.B_LOC, c.V], F32, isOutput=True)

    dbgout = None
    if dbg:
        dbgout = {
            "dG": nc.declare_dram_parameter("dG", [P, c.SPP * c.EC], F32, isOutput=True),
            "duT": nc.declare_dram_parameter("duT", [P, c.B_LOC], F32, isOutput=True),
        }
    with tile.TileContext(nc) as tc:
        _body(c, nc, tc, story, quest, embcat, emb3T, bmask_bf, bmask2, out, dbgout)
    return nc


def _tree_sum(nc, scratch, src, dst, np_, ec):
    """dst[:, 0, :] = sum over the 50-token axis of src [np_, 50, ec].

    Unit-stride halving adds into a shared bf16 scratch tile [P, 48, ec].
    First level split into two ops so no single DVE op blocks the gather
    stream for long. src may be fp8; scratch/dst are bf16.
    """
    s_ = lambda a, b: scratch[:np_, a:b, :]
    # a1[i] = src[i] + src[13+i], i<13 -> cols 0:13; a2 covers 26..49 -> 13:25
    nc.vector.tensor_add(out=s_(0, 13), in0=src[:np_, 0:13, :], in1=src[:np_, 13:26, :])
    nc.vector.tensor_add(out=s_(13, 25), in0=src[:np_, 26:38, :], in1=src[:np_, 38:50, :])
    nc.vector.tensor_add(out=s_(25, 31), in0=s_(0, 6), in1=s_(6, 12))
    nc.vector.tensor_add(out=s_(31, 37), in0=s_(13, 19), in1=s_(19, 25))
    nc.vector.tensor_add(out=s_(37, 40), in0=s_(25, 28), in1=s_(28, 31))
    nc.vector.tensor_add(out=s_(40, 43), in0=s_(31, 34), in1=s_(34, 37))
    nc.vector.tensor_add(out=s_(43, 46), in0=s_(37, 40), in1=s_(40, 43))
    nc.vector.tensor_add(out=s_(46, 47), in0=s_(43, 44), in1=s_(44, 45))
    nc.vector.tensor_add(out=s_(47, 48), in0=s_(46, 47), in1=s_(45, 46))
    # + leftover a1[12] (holds src[12] + src[25])
    nc.vector.tensor_add(out=dst, in0=s_(47, 48), in1=s_(12, 13))


def _body(c: Cfg, nc, tc, story, quest, embcat, emb3T, bmask_bf, bmask2, out, dbgout=None):
    with ExitStack() as es:
        # ---------- persistent pools ----------
        cpool = es.enter_context(tc.tile_pool(name="const", bufs=1))
        gpool = es.enter_context(tc.tile_pool(name="G", bufs=1))
        upool = es.enter_context(tc.tile_pool(name="u", bufs=1))

        identity = cpool.tile([P, P], F32)
        make_identity(nc, identity[:])
        identity_bf = cpool.tile([P, P], BF16)
        nc.vector.tensor_copy(out=identity_bf[:], in_=identity[:])

        # story indices resident in SBUF: [128, SPP*SENT]
        idx_t = cpool.tile([P, c.SPP * c.SENT], I32)
        nc.sync.dma_start(
            out=idx_t[:],
            in_=story[:].rearrange("(p j) t -> p (j t)", p=P),
        )
        qidx_t = cpool.tile([P, c.QI], I32)
        nc.sync.dma_start(out=qidx_t[:], in_=quest[:])
        bmask_t = cpool.tile([P, c.B_LOC], BF16)
        nc.sync.dma_start(out=bmask_t[:], in_=bmask_bf[:])
        bmask2_t = cpool.tile([P, c.B_LOC], F32)
        nc.sync.dma_start(out=bmask2_t[:], in_=bmask2[:])

        # G concat table [128, SPP, 4E] bf16; G_t = G[:, :, t*E:(t+1)*E]
        G = gpool.tile([P, c.SPP, c.EC], BF16, tag="G", name="G")
        # G^T for m-tables [E=128, TOT_SLOTS]
        GT = [gpool.tile([P, c.TOT_SLOTS], BF16, tag=f"GT{t}", name=f"GT{t}") for t in range(c.K_HOP)]

        # ---------- gather + segment-reduce ----------
        GDT = F8 if GATHER_FP8 else BF16
        with (
            tc.tile_pool(name="gather", bufs=3 if GATHER_FP8 else 2) as gbpool,
            tc.tile_pool(name="red", bufs=1) as rpool,
        ):
            scratch = rpool.tile([P, 48, c.EC], BF16, tag="scr")
            for j in range(c.SPP):
                gbuf = gbpool.tile([P, c.SENT, c.EC], GDT, tag="gbuf")
                # NB: HW SWDGE reads ONE offset per partition per instruction
                # (extra offset-AP columns are ignored and the free dim is
                # filled with consecutive rows — probed); so one instruction
                # per token column, 2D out slice, exactly like the baseline.
                for s in range(c.SENT):
                    nc.gpsimd.indirect_dma_start(
                        out=gbuf[:, s, :],
                        out_offset=None,
                        in_=embcat[:],
                        in_offset=bass.IndirectOffsetOnAxis(
                            ap=idx_t[:, j * c.SENT + s : j * c.SENT + s + 1],
                            axis=0,
                        ),
                    )
                _tree_sum(nc, scratch, gbuf, G[:, j : j + 1, :], P, c.EC)
            # question gather: [128, QI] indices -> per-partition partial sums
            # q4 [128, EC]; per-batch token sum finished on PE via bmask
            qbuf = gbpool.tile([P, c.SENT, c.EC], GDT, tag="gbuf")
            for s in range(c.QI):
                nc.gpsimd.indirect_dma_start(
                    out=qbuf[:, s, :],
                    out_offset=None,
                    in_=embcat[:],
                    in_offset=bass.IndirectOffsetOnAxis(
                        ap=qidx_t[:, s : s + 1], axis=0
                    ),
                )
            q4 = upool.tile([P, c.EC], BF16, tag="q4")
            assert c.QI == 4
            nc.vector.tensor_add(
                out=scratch[:, 0:1, :], in0=qbuf[:, 0:1, :], in1=qbuf[:, 1:2, :]
            )
            nc.vector.tensor_add(
                out=scratch[:, 1:2, :], in0=qbuf[:, 2:3, :], in1=qbuf[:, 3:4, :]
            )
            nc.vector.tensor_add(
                out=q4[:].unsqueeze(1), in0=scratch[:, 0:1, :], in1=scratch[:, 1:2, :]
            )

        # ---------- transposes: GT_t from G_t; uT0 from u0 ----------
        with tc.tile_pool(name="tp", bufs=4, space="PSUM") as tppool:
            for t in range(c.K_HOP):
                for j in range(c.SPP):
                    tp = tppool.tile([P, P], F32, tag="tp")
                    nc.tensor.matmul(
                        out=tp[:], lhsT=G[:, j, t * E : (t + 1) * E], rhs=identity_bf[:],
                        start=True, stop=True,
                    )
                    # psum col p <-> slot 13p+j: write GT[:, j::SPP]
                    nc.vector.tensor_copy(
                        out=GT[t][:].rearrange("e (p j) -> e p j", j=c.SPP)[:, :, j],
                        in_=tp[:],
                    )
            # uT[e, b] = sum_p q4[p, e] * bmask[p, b]  (finishes the question
            # token sum across the PPB partitions of each batch, pre-transposed)
            uT = upool.tile([P, c.B_LOC], F32, tag="uT")
            uT_bf = upool.tile([P, c.B_LOC], BF16, tag="uT_bf")
            tpu = tppool.tile([P, c.B_LOC], F32, tag="tpu")
            nc.tensor.matmul(
                out=tpu[:], lhsT=q4[:, :E], rhs=bmask_t[:],
                start=True, stop=True,
            )
            nc.vector.tensor_copy(out=uT[:], in_=tpu[:])
            nc.vector.tensor_copy(out=uT_bf[:], in_=tpu[:])

        if dbgout is not None:
            # gpsimd dma casts bf16 -> f32 on the way out
            nc.gpsimd.dma_start(
                out=dbgout["dG"][:],
                in_=G[:].rearrange("p a b -> p (a b)"),
            )
            nc.sync.dma_start(out=dbgout["duT"][:], in_=uT[:])

        # ---------- K_HOP attention hops ----------
        with (
            tc.tile_pool(name="hop", bufs=2) as hpool,
            tc.tile_pool(name="hop_ps", bufs=1, space="PSUM") as hpspool,
            tc.tile_pool(name="hop_ps2", bufs=2, space="PSUM") as hpspool2,
        ):
            for h in range(c.K_HOP):
                # scores [B_LOC, TOT_SLOTS] = uT.T @ GT[h]
                sc_ps = hpspool.tile([c.B_LOC, c.TOT_SLOTS], F32, tag="sc")
                for c0 in range(0, c.TOT_SLOTS, 512):
                    c1 = min(c0 + 512, c.TOT_SLOTS)
                    nc.tensor.matmul(
                        out=sc_ps[:, c0:c1],
                        lhsT=uT_bf[:],
                        rhs=GT[h][:, c0:c1],
                        start=True,
                        stop=True,
                    )
                # move scores to SBUF, bounce via DRAM with a diagonal AP to
                # get per-batch aligned scores scal[b, s] = scores[b, S_PAD*b + s]
                sc_sb = hpool.tile([c.B_LOC, c.TOT_SLOTS], F32, tag="sc_sb")
                nc.vector.tensor_copy(out=sc_sb[:], in_=sc_ps[:])
                scd = nc.dram_tensor(f"scd{h}", [c.B_LOC * c.TOT_SLOTS], F32)
                nc.sync.dma_start(
                    out=scd[:].rearrange("(b t) -> b t", t=c.TOT_SLOTS), in_=sc_sb[:]
                )
                diag = bass.AP(
                    tensor=scd[:].tensor,
                    offset=0,
                    ap=[[c.TOT_SLOTS + c.S_PAD, c.B_LOC], [1, c.S_PAD]],
                )
                scal = hpool.tile([c.B_LOC, c.S_PAD], F32, tag="scal")
                nc.sync.dma_start(out=scal[:], in_=diag)
                # masked softmax over the S real sentences
                probs = hpool.tile([c.B_LOC, c.S_PAD], F32, tag="probs")
                nc.vector.memset(probs[:], 0.0)
                negmax = hpool.tile([c.B_LOC, 1], F32, tag="negmax")
                nc.vector.tensor_reduce(
                    out=negmax[:], in_=scal[:, : c.S], axis=AX.X, op=ALU.max, negate=True
                )
                denom = hpool.tile([c.B_LOC, 1], F32, tag="denom")
                nc.scalar.activation(
                    out=probs[:, : c.S],
                    in_=scal[:, : c.S],
                    func=ACTF.Exp,
                    bias=negmax[:],
                    scale=1.0,
                    accum_out=denom[:],
                )
                rec = hpool.tile([c.B_LOC, 1], F32, tag="rec")
                nc.vector.reciprocal(out=rec[:], in_=denom[:])
                nc.vector.tensor_scalar_mul(probs[:, : c.S], probs[:, : c.S], rec[:])
                # repack probs [B_LOC, S_PAD] -> slot layout [128, SPP] via DRAM bounce
                pd = nc.dram_tensor(f"pd{h}", [c.TOT_SLOTS], F32)
                nc.sync.dma_start(
                    out=pd[:].rearrange("(b s) -> b s", s=c.S_PAD), in_=probs[:]
                )
                pslot = hpool.tile([P, c.SPP], F32, tag="pslot")
                nc.sync.dma_start(
                    out=pslot[:], in_=pd[:].rearrange("(p j) -> p j", j=c.SPP)
                )
                pslot_bf = hpool.tile([P, c.SPP], BF16, tag="pslot_bf")
                nc.vector.tensor_copy(out=pslot_bf[:], in_=pslot[:])
                # block-diagonal probs [128, SPP, B_LOC] = pslot (bcast) * bmask (bcast)
                bd = hpool.tile([P, c.SPP, c.B_LOC], BF16, tag="bd")
                nc.vector.tensor_tensor(
                    out=bd[:],
                    in0=pslot_bf[:].unsqueeze(-1).to_broadcast([P, c.SPP, c.B_LOC]),
                    in1=bmask_t[:].unsqueeze(1).to_broadcast([P, c.SPP, c.B_LOC]),
                    op=ALU.mult,
                )
                # combine: uT_new = sum_j G[h+1][:,j,:].T @ bd[:,j,:]  (+ uT)
                uc_ps = hpspool2.tile([P, c.B_LOC], F32, tag="uc")
                for j in range(c.SPP):
                    nc.tensor.matmul(
                        out=uc_ps[:],
                        lhsT=G[:, j, (h + 1) * E : (h + 2) * E],
                        rhs=bd[:, j, :],
                        start=(j == 0),
                        stop=(j == c.SPP - 1),
                    )
                uT_new = upool.tile([P, c.B_LOC], F32, tag=f"uT{h + 1}")
                nc.vector.tensor_add(out=uT_new[:], in0=uc_ps[:], in1=uT[:])
                uT = uT_new
                uT_bf_new = upool.tile([P, c.B_LOC], BF16, tag=f"uT_bf{h + 1}")
                nc.vector.tensor_copy(out=uT_bf_new[:], in_=uT[:])
                uT_bf = uT_bf_new

        # ---------- final phase: logits + vocab softmax ----------
        with (
            tc.tile_pool(name="fin", bufs=1) as fpool,
            tc.tile_pool(name="emb3c", bufs=2) as epool,
            tc.tile_pool(name="fin_ps", bufs=2, space="PSUM") as fps,
            tc.tile_pool(name="den_ps", bufs=1, space="PSUM") as dps,
            tc.tile_pool(name="out_ps", bufs=4, space="PSUM") as ops,
            tc.tile_pool(name="outsb", bufs=4) as osb,
        ):
            ones = fpool.tile([P, P], F32)
            nc.vector.memset(ones[:], 1.0)
            ones_part = fpool.tile([P, P], F32)
            nc.vector.memset(ones_part[:], 0.0)
            nc.vector.memset(ones_part[: c.LAST_VT_ROWS, :], 1.0)

            exp_buf = fpool.tile([P, c.NVT * c.B_LOC], F32)
            CW = c.CHUNK_VT * c.B_LOC  # psum/exp cols per chunk
            den_ps = dps.tile([P, CW], F32)
            for ch in range(c.NCH):
                vt0 = ch * c.CHUNK_VT
                nvt = min(c.CHUNK_VT, c.NVT - vt0)
                echunk = epool.tile([P, c.CHUNK_VT * P], BF16, tag="echunk")
                nc.sync.dma_start(
                    out=echunk[:, : nvt * P],
                    in_=emb3T[:, vt0 * P : (vt0 + nvt) * P],
                )
                lg_ps = fps.tile([P, CW], F32, tag="lg")
                for m in range(nvt):
                    nc.tensor.matmul(
                        out=lg_ps[:, m * c.B_LOC : (m + 1) * c.B_LOC],
                        lhsT=echunk[:, m * P : (m + 1) * P],
                        rhs=uT_bf[:],
                        start=True,
                        stop=True,
                    )
                ecols = nvt * c.B_LOC
                nc.scalar.activation(
                    out=exp_buf[:, vt0 * c.B_LOC : vt0 * c.B_LOC + ecols],
                    in_=lg_ps[:, :ecols],
                    func=ACTF.Exp,
                )
                # denominator partials: ones^T @ exp_chunk, accumulated in psum
                exp_ch = exp_buf[:, vt0 * c.B_LOC : vt0 * c.B_LOC + ecols]
                last_has_partial = vt0 + nvt == c.NVT and c.LAST_VT_ROWS < P
                full_cols = ecols - (c.B_LOC if last_has_partial else 0)
                if full_cols > 0:
                    nc.tensor.matmul(
                        out=den_ps[:, :full_cols],
                        lhsT=ones[:],
                        rhs=exp_ch[:, :full_cols],
                        start=(ch == 0),
                        stop=False,
                        skip_group_check=True,
                    )
                if last_has_partial:
                    nc.tensor.matmul(
                        out=den_ps[:, full_cols:ecols],
                        lhsT=ones_part[:],
                        rhs=exp_ch[:, full_cols:ecols],
                        start=False,
                        stop=True,
                        skip_group_check=True,
                    )
            # denominators [1, B_LOC] then reciprocal replicated to [128,1]
            den8 = fpool.tile([P, c.B_LOC], F32)
            nc.vector.tensor_reduce(
                out=den8[:].unsqueeze(-1),
                in_=den_ps[:].rearrange("o (m b) -> o b m", b=c.B_LOC),
                axis=AX.X,
                op=ALU.add,
            )
            rec8 = fpool.tile([P, c.B_LOC], F32)
            nc.vector.reciprocal(out=rec8[:], in_=den8[:])
            # rec_rep[p] = rec8[p % B_LOC] via mask multiply + free reduce
            rec_full = fpool.tile([P, c.B_LOC], F32)
            nc.vector.tensor_tensor(
                out=rec_full[:],
                in0=bmask2_t[:],
                in1=rec8[:],
                op=ALU.mult,
            )
            rec_rep = fpool.tile([P, 1], F32)
            nc.vector.tensor_reduce(
                out=rec_rep[:], in_=rec_full[:], axis=AX.X, op=ALU.add
            )

            # transpose 16-V-tile groups, scale by recip, DMA out
            GRP = P // c.B_LOC  # V tiles per transpose group
            ngrp = -(-c.NVT // GRP)
            n_full_vt = c.V // P  # V tiles fully inside the real vocab
            out3 = out[:, : n_full_vt * P].rearrange("b (t col) -> t b col", col=P)
            for g in range(ngrp):
                t0 = g * GRP
                nt = min(GRP, c.NVT - t0)
                cols = nt * c.B_LOC
                tps = ops.tile([P, P], F32, tag="otp")
                nc.tensor.matmul(
                    out=tps[:cols, :],
                    lhsT=exp_buf[:, t0 * c.B_LOC : t0 * c.B_LOC + cols],
                    rhs=identity[:],
                    start=True,
                    stop=True,
                )
                sb = osb.tile([P, P], F32, tag="osb")
                nc.vector.tensor_scalar_mul(sb[:cols, :], tps[:cols, :], rec_rep[:cols, :])
                # rows b + B_LOC*t', t' = local V-tile; tail V tile may be partial
                full_t = min(nt, n_full_vt - t0)
                if full_t > 0:
                    nc.sync.dma_start(
                        out=out3[t0 : t0 + full_t],
                        in_=sb[: full_t * c.B_LOC, :],
                    )
                if full_t < nt:  # partial last V tile
                    nc.sync.dma_start(
                        out=out[:, n_full_vt * P : c.V],
                        in_=sb[full_t * c.B_LOC : cols, : c.V - n_full_vt * P],
                    )


# ---------------- host-side pack/unpack ----------------
def ref_numpy(story, question, emb_A):
    """Full-batch numpy reference (mirrors reference.py)."""
    K_HOP = emb_A.shape[0] - 1
    u = emb_A[0][question].sum(axis=1)
    for i in range(K_HOP):
        m = emb_A[i][story].sum(axis=2)
        cc = emb_A[i + 1][story].sum(axis=2)
        logits_att = np.einsum("bse,be->bs", m, u)
        pa = np.exp(logits_att - logits_att.max(-1, keepdims=True))
        probs = pa / pa.sum(-1, keepdims=True)
        u = np.einsum("bse,bs->be", cc, probs) + u
    logits = u @ emb_A[-1].T
    z = np.exp(logits - logits.max(-1, keepdims=True))
    return (z / z.sum(-1, keepdims=True)).astype(np.float32)


N_CORES = 8
_CACHE = {}


def _get_nc(cfg):
    key = "nc"
    if key not in _CACHE:
        import concourse.bacc as bacc

        nc = bacc.Bacc(target_bir_lowering=False)
        build_kernel(cfg, nc)
        nc.finalize()
        _CACHE[key] = nc
    return _CACHE[key]


def _pack_shared(cfg, emb_A):
    key = "shared"
    if key not in _CACHE or _CACHE[key][0] is not emb_A:
        c = cfg
        import ml_dtypes

        embs = {}
        # concat tables [V, NT*E] + zero pad row
        ecat = np.zeros((c.V + 1, c.EC), np.float32)
        ecat[: c.V] = np.concatenate([emb_A[t] for t in range(c.NT)], axis=1)
        gdt = ml_dtypes.float8_e4m3 if GATHER_FP8 else ml_dtypes.bfloat16
        embs["embcat"] = ecat.astype(gdt)
        e3T = np.zeros((E, c.VPAD), np.float32)
        e3T[:, : c.V] = emb_A[c.NT - 1].T
        embs["emb3T"] = e3T.astype(ml_dtypes.bfloat16)
        bm = np.zeros((P, c.B_LOC), np.float32)
        for b in range(c.B_LOC):
            bm[b * c.PPB : (b + 1) * c.PPB, b] = 1.0
        embs["bmask_bf"] = bm.astype(ml_dtypes.bfloat16)
        bm2 = np.zeros((P, c.B_LOC), np.float32)
        for p in range(P):
            bm2[p, p % c.B_LOC] = 1.0
        embs["bmask2"] = bm2
        _CACHE[key] = (emb_A, embs)
    return _CACHE[key][1]


def _pack_story(cfg, story_c):
    c = cfg
    story_pad = np.full((c.B_LOC, c.S_PAD, c.SENT), c.V, np.int32)
    story_pad[:, : c.S, :] = story_c
    return np.ascontiguousarray(story_pad.reshape(c.TOT_SLOTS, c.SENT))


def _pack_question(cfg, quest_c):
    """[B_LOC, SENT] -> [128, QI]: partition PPB*b + r holds tokens
    QI*r .. QI*r+QI-1 of batch b (pads -> V)."""
    c = cfg
    qp = np.full((c.B_LOC, c.PPB * c.QI), c.V, np.int32)
    qp[:, : c.SENT] = quest_c
    return np.ascontiguousarray(qp.reshape(P, c.QI))


def kernel(story, question, emb_A, _trace=False, _trace_kwargs=None):
    from concourse import bass_utils

    story = np.asarray(story)
    question = np.asarray(question)
    emb_A = np.asarray(emb_A)

    cfg = Cfg(
        B_LOC=story.shape[0] // N_CORES,
        S=story.shape[1],
        SENT=story.shape[2],
        V=emb_A.shape[1],
        K_HOP=emb_A.shape[0] - 1,
    )
    nc = _get_nc(cfg)
    shared = _pack_shared(cfg, emb_A)
    in_maps = []
    for ci in range(N_CORES):
        sl = slice(ci * cfg.B_LOC, (ci + 1) * cfg.B_LOC)
        in_maps.append(
            {
                "story_pad": _pack_story(cfg, story[sl]),
                "question": _pack_question(cfg, np.asarray(question[sl]).astype(np.int32)),
                **shared,
            }
        )
    kwargs = {}
    if _trace:
        kwargs = dict(trace=True, trace_kwargs=_trace_kwargs or {})
    res = bass_utils.run_bass_kernel_spmd(
        nc, in_maps, core_ids=list(range(N_CORES)), **kwargs
    )
    out = np.concatenate([r["out"] for r in res.results], axis=0)
    if _trace:
        return out, res
    return out


# revision 35
# speedup vs baseline: 1.2779x; 1.0302x over previous
"""MemN2N Bass kernel builder (per-core program, SPMD over 8 cores).

Per-core work (core c):
  - 8 local batches (B_LOC). story_pad [TOT_SLOTS, SENT] int32 staged so that
    slot(16b+q, j) = batch b, sentence 13q+j  (S_PAD sents/batch, SPP per part).
    Pad tokens point at table row V (a zero row appended host-side).
  - The 4 tables are concatenated host-side into embcat [V+1, 4E] bf16 so one
    gathered row covers all tables (4x fewer gather instructions than
    per-table). HW indirect DMA reads ONE offset per partition per
    instruction, so it takes SENT*SPP=650 instructions (~1.66us each, SWDGE
    fixed-cost bound).
  - Token-sum via unit-stride halving adds on DVE -> G [128, SPP, 4E] bf16,
    overlapped under the gather stream (ops kept <= 13 cols so none blocks
    the gathers for long).
  - 3 attention hops (PE transposes for G^T, scores matmul, softmax on ACT,
    DRAM-bounce repack, block-diag combine matmul) -> u3.
  - logits = u3 @ emb3.T via emb3T bf16 [E, VPAD] staged pre-transposed;
    softmax over vocab computed on-device; output [B_LOC, V] f32.
"""
import sys

sys.path.insert(0, "/opt/trn_rl_repo")

from contextlib import ExitStack

import numpy as np

import concourse.bass as bass
import concourse.mybir as mybir
import concourse.tile as tile
from concourse.masks import make_identity

F32 = mybir.dt.float32
BF16 = mybir.dt.bfloat16
F8 = mybir.dt.float8e4
I32 = mybir.dt.int32
GATHER_FP8 = False  # e4m3 embcat measured slower (DVE fp8 adds) and noisier
FINAL_FP8 = True  # e4m3 emb3T + uT: halves final-phase weight stream, faster LDWEIGHTS
AX = mybir.AxisListType
ALU = mybir.AluOpType
ACTF = mybir.ActivationFunctionType

P = 128
E = 128


class Cfg:
    def __init__(self, B_LOC=8, S=200, SENT=50, V=100000, K_HOP=3, CHUNK_VT=32):
        self.B_LOC = B_LOC
        self.S = S
        self.SENT = SENT
        self.V = V
        self.K_HOP = K_HOP
        self.NT = K_HOP + 1
        self.EC = self.NT * E  # concat row width
        self.PPB = P // B_LOC  # partitions per batch
        self.QI = -(-SENT // self.PPB)  # question tokens per partition
        # S_PAD: sentences per batch padded so B_LOC*S_PAD = 128*SPP
        self.SPP = -(-(B_LOC * S) // P)  # ceil
        self.S_PAD = self.PPB * self.SPP
        assert self.S_PAD >= S
        self.TOT_SLOTS = P * self.SPP
        # vocab padding for 128-row tiles
        self.NVT = -(-V // P)  # number of V tiles
        self.VPAD = self.NVT * P
        self.LAST_VT_ROWS = V - (self.NVT - 1) * P  # valid rows in last V tile
        # final-phase chunking: CHUNK_VT V-tiles of logits per psum/exp chunk
        self.CHUNK_VT = CHUNK_VT
        self.NCH = -(-self.NVT // CHUNK_VT)


def build_kernel(cfg: Cfg, nc: bass.Bass, dbg: bool = False):
    c = cfg
    # ---- I/O ----
    story = nc.declare_dram_parameter("story_pad", [c.TOT_SLOTS, c.SENT], I32, isOutput=False)
    # question tokens packed [128, QI]: partition PPB*b + r holds tokens
    # QI*r .. QI*r+QI-1 of batch b (pads -> row V)
    quest = nc.declare_dram_parameter("question", [P, c.QI], I32, isOutput=False)
    embcat = nc.declare_dram_parameter(
        "embcat", [c.V + 1, c.EC], F8 if GATHER_FP8 else BF16, isOutput=False
    )
    emb3T = nc.declare_dram_parameter(
        "emb3T", [E, c.VPAD], F8 if FINAL_FP8 else BF16, isOutput=False
    )
    bmask_bf = nc.declare_dram_parameter("bmask_bf", [P, c.B_LOC], BF16, isOutput=False)
    bmask2 = nc.declare_dram_parameter("bmask2", [P, c.B_LOC], F32, isOutput=False)
    out = nc.declare_dram_parameter("out", [c.B_LOC, c.V], F32, isOutput=True)

    dbgout = None
    if dbg:
        dbgout = {
            "dG": nc.declare_dram_parameter("dG", [P, c.SPP * c.EC], F32, isOutput=True),
            "duT": nc.declare_dram_parameter("duT", [P, c.B_LOC], F32, isOutput=True),
        }
    with tile.TileContext(nc) as tc:
        _body(c, nc, tc, story, quest, embcat, emb3T, bmask_bf, bmask2, out, dbgout)
    return nc


def _tree_sum(nc, scratch, src, dst, np_, ec):
    """dst[:, 0, :] = sum over the 50-token axis of src [np_, 50, ec].

    Unit-stride halving adds into a shared bf16 scratch tile [P, 48, ec].
    First level split into two ops so no single DVE op blocks the gather
    stream for long. src may be fp8; scratch/dst are bf16.
    """
    s_ = lambda a, b: scratch[:np_, a:b, :]
    # a1[i] = src[i] + src[13+i], i<13 -> cols 0:13; a2 covers 26..49 -> 13:25
    nc.vector.tensor_add(out=s_(0, 13), in0=src[:np_, 0:13, :], in1=src[:np_, 13:26, :])
    nc.vector.tensor_add(out=s_(13, 25), in0=src[:np_, 26:38, :], in1=src[:np_, 38:50, :])
    nc.vector.tensor_add(out=s_(25, 31), in0=s_(0, 6), in1=s_(6, 12))
    nc.vector.tensor_add(out=s_(31, 37), in0=s_(13, 19), in1=s_(19, 25))
    nc.vector.tensor_add(out=s_(37, 40), in0=s_(25, 28), in1=s_(28, 31))
    nc.vector.tensor_add(out=s_(40, 43), in0=s_(31, 34), in1=s_(34, 37))
    nc.vector.tensor_add(out=s_(43, 46), in0=s_(37, 40), in1=s_(40, 43))
    nc.vector.tensor_add(out=s_(46, 47), in0=s_(43, 44), in1=s_(44, 45))
    nc.vector.tensor_add(out=s_(47, 48), in0=s_(46, 47), in1=s_(45, 46))
    # + leftover a1[12] (holds src[12] + src[25])
    nc.vector.tensor_add(out=dst, in0=s_(47, 48), in1=s_(12, 13))


def _body(c: Cfg, nc, tc, story, quest, embcat, emb3T, bmask_bf, bmask2, out, dbgout=None):
    with ExitStack() as es:
        # ---------- persistent pools ----------
        cpool = es.enter_context(tc.tile_pool(name="const", bufs=1))
        gpool = es.enter_context(tc.tile_pool(name="G", bufs=1))
        upool = es.enter_context(tc.tile_pool(name="u", bufs=1))

        identity = cpool.tile([P, P], F32)
        make_identity(nc, identity[:])
        identity_bf = cpool.tile([P, P], BF16)
        nc.vector.tensor_copy(out=identity_bf[:], in_=identity[:])

        # story indices resident in SBUF: [128, SPP*SENT]
        idx_t = cpool.tile([P, c.SPP * c.SENT], I32)
        nc.sync.dma_start(
            out=idx_t[:],
            in_=story[:].rearrange("(p j) t -> p (j t)", p=P),
        )
        qidx_t = cpool.tile([P, c.QI], I32)
        nc.sync.dma_start(out=qidx_t[:], in_=quest[:])
        bmask_t = cpool.tile([P, c.B_LOC], BF16)
        nc.sync.dma_start(out=bmask_t[:], in_=bmask_bf[:])
        bmask2_t = cpool.tile([P, c.B_LOC], F32)
        nc.sync.dma_start(out=bmask2_t[:], in_=bmask2[:])

        # G concat table [128, SPP, 4E] bf16; G_t = G[:, :, t*E:(t+1)*E]
        G = gpool.tile([P, c.SPP, c.EC], BF16, tag="G", name="G")
        # G^T for m-tables [E=128, TOT_SLOTS]
        GT = [gpool.tile([P, c.TOT_SLOTS], BF16, tag=f"GT{t}", name=f"GT{t}") for t in range(c.K_HOP)]

        # ---------- gather + segment-reduce ----------
        GDT = F8 if GATHER_FP8 else BF16
        with (
            tc.tile_pool(name="gather", bufs=3 if GATHER_FP8 else 2) as gbpool,
            tc.tile_pool(name="red", bufs=1) as rpool,
        ):
            scratch = rpool.tile([P, 48, c.EC], BF16, tag="scr")
            for j in range(c.SPP):
                gbuf = gbpool.tile([P, c.SENT, c.EC], GDT, tag="gbuf")
                # NB: HW SWDGE reads ONE offset per partition per instruction
                # (extra offset-AP columns are ignored and the free dim is
                # filled with consecutive rows — probed); so one instruction
                # per token column, 2D out slice, exactly like the baseline.
                for s in range(c.SENT):
                    nc.gpsimd.indirect_dma_start(
                        out=gbuf[:, s, :],
                        out_offset=None,
                        in_=embcat[:],
                        in_offset=bass.IndirectOffsetOnAxis(
                            ap=idx_t[:, j * c.SENT + s : j * c.SENT + s + 1],
                            axis=0,
                        ),
                    )
                _tree_sum(nc, scratch, gbuf, G[:, j : j + 1, :], P, c.EC)
            # question gather: [128, QI] indices -> per-partition partial sums
            # q4 [128, EC]; per-batch token sum finished on PE via bmask
            qbuf = gbpool.tile([P, c.SENT, c.EC], GDT, tag="gbuf")
            for s in range(c.QI):
                nc.gpsimd.indirect_dma_start(
                    out=qbuf[:, s, :],
                    out_offset=None,
                    in_=embcat[:],
                    in_offset=bass.IndirectOffsetOnAxis(
                        ap=qidx_t[:, s : s + 1], axis=0
                    ),
                )
            q4 = upool.tile([P, c.EC], BF16, tag="q4")
            assert c.QI == 4
            nc.vector.tensor_add(
                out=scratch[:, 0:1, :], in0=qbuf[:, 0:1, :], in1=qbuf[:, 1:2, :]
            )
            nc.vector.tensor_add(
                out=scratch[:, 1:2, :], in0=qbuf[:, 2:3, :], in1=qbuf[:, 3:4, :]
            )
            nc.vector.tensor_add(
                out=q4[:].unsqueeze(1), in0=scratch[:, 0:1, :], in1=scratch[:, 1:2, :]
            )

        # ---------- transposes: GT_t from G_t; uT0 from u0 ----------
        with tc.tile_pool(name="tp", bufs=4, space="PSUM") as tppool:
            for t in range(c.K_HOP):
                for j in range(c.SPP):
                    tp = tppool.tile([P, P], F32, tag="tp")
                    nc.tensor.matmul(
                        out=tp[:], lhsT=G[:, j, t * E : (t + 1) * E], rhs=identity_bf[:],
                        start=True, stop=True,
                    )
                    # psum col p <-> slot 13p+j: write GT[:, j::SPP]
                    nc.vector.tensor_copy(
                        out=GT[t][:].rearrange("e (p j) -> e p j", j=c.SPP)[:, :, j],
                        in_=tp[:],
                    )
            # uT[e, b] = sum_p q4[p, e] * bmask[p, b]  (finishes the question
            # token sum across the PPB partitions of each batch, pre-transposed)
            uT = upool.tile([P, c.B_LOC], F32, tag="uT")
            uT_bf = upool.tile([P, c.B_LOC], BF16, tag="uT_bf")
            tpu = tppool.tile([P, c.B_LOC], F32, tag="tpu")
            nc.tensor.matmul(
                out=tpu[:], lhsT=q4[:, :E], rhs=bmask_t[:],
                start=True, stop=True,
            )
            nc.vector.tensor_copy(out=uT[:], in_=tpu[:])
            nc.vector.tensor_copy(out=uT_bf[:], in_=tpu[:])

        if dbgout is not None:
            # gpsimd dma casts bf16 -> f32 on the way out
            nc.gpsimd.dma_start(
                out=dbgout["dG"][:],
                in_=G[:].rearrange("p a b -> p (a b)"),
            )
            nc.sync.dma_start(out=dbgout["duT"][:], in_=uT[:])

        # ---------- K_HOP attention hops ----------
        with (
            tc.tile_pool(name="hop", bufs=2) as hpool,
            tc.tile_pool(name="hop_ps", bufs=1, space="PSUM") as hpspool,
            tc.tile_pool(name="hop_ps2", bufs=2, space="PSUM") as hpspool2,
        ):
            for h in range(c.K_HOP):
                # scores [B_LOC, TOT_SLOTS] = uT.T @ GT[h]
                sc_ps = hpspool.tile([c.B_LOC, c.TOT_SLOTS], F32, tag="sc")
                for c0 in range(0, c.TOT_SLOTS, 512):
                    c1 = min(c0 + 512, c.TOT_SLOTS)
                    nc.tensor.matmul(
                        out=sc_ps[:, c0:c1],
                        lhsT=uT_bf[:],
                        rhs=GT[h][:, c0:c1],
                        start=True,
                        stop=True,
                    )
                # move scores to SBUF, bounce via DRAM with a diagonal AP to
                # get per-batch aligned scores scal[b, s] = scores[b, S_PAD*b + s]
                sc_sb = hpool.tile([c.B_LOC, c.TOT_SLOTS], F32, tag="sc_sb")
                nc.vector.tensor_copy(out=sc_sb[:], in_=sc_ps[:])
                scd = nc.dram_tensor(f"scd{h}", [c.B_LOC * c.TOT_SLOTS], F32)
                nc.sync.dma_start(
                    out=scd[:].rearrange("(b t) -> b t", t=c.TOT_SLOTS), in_=sc_sb[:]
                )
                diag = bass.AP(
                    tensor=scd[:].tensor,
                    offset=0,
                    ap=[[c.TOT_SLOTS + c.S_PAD, c.B_LOC], [1, c.S_PAD]],
                )
                scal = hpool.tile([c.B_LOC, c.S_PAD], F32, tag="scal")
                nc.sync.dma_start(out=scal[:], in_=diag)
                # masked softmax over the S real sentences
                probs = hpool.tile([c.B_LOC, c.S_PAD], F32, tag="probs")
                nc.vector.memset(probs[:], 0.0)
                negmax = hpool.tile([c.B_LOC, 1], F32, tag="negmax")
                nc.vector.tensor_reduce(
                    out=negmax[:], in_=scal[:, : c.S], axis=AX.X, op=ALU.max, negate=True
                )
                denom = hpool.tile([c.B_LOC, 1], F32, tag="denom")
                nc.scalar.activation(
                    out=probs[:, : c.S],
                    in_=scal[:, : c.S],
                    func=ACTF.Exp,
                    bias=negmax[:],
                    scale=1.0,
                    accum_out=denom[:],
                )
                rec = hpool.tile([c.B_LOC, 1], F32, tag="rec")
                nc.vector.reciprocal(out=rec[:], in_=denom[:])
                nc.vector.tensor_scalar_mul(probs[:, : c.S], probs[:, : c.S], rec[:])
                # repack probs [B_LOC, S_PAD] -> slot layout [128, SPP] via DRAM bounce
                pd = nc.dram_tensor(f"pd{h}", [c.TOT_SLOTS], F32)
                nc.sync.dma_start(
                    out=pd[:].rearrange("(b s) -> b s", s=c.S_PAD), in_=probs[:]
                )
                pslot = hpool.tile([P, c.SPP], F32, tag="pslot")
                nc.sync.dma_start(
                    out=pslot[:], in_=pd[:].rearrange("(p j) -> p j", j=c.SPP)
                )
                pslot_bf = hpool.tile([P, c.SPP], BF16, tag="pslot_bf")
                nc.vector.tensor_copy(out=pslot_bf[:], in_=pslot[:])
                # block-diagonal probs [128, SPP, B_LOC] = pslot (bcast) * bmask (bcast)
                bd = hpool.tile([P, c.SPP, c.B_LOC], BF16, tag="bd")
                nc.vector.tensor_tensor(
                    out=bd[:],
                    in0=pslot_bf[:].unsqueeze(-1).to_broadcast([P, c.SPP, c.B_LOC]),
                    in1=bmask_t[:].unsqueeze(1).to_broadcast([P, c.SPP, c.B_LOC]),
                    op=ALU.mult,
                )
                # combine: uT_new = sum_j G[h+1][:,j,:].T @ bd[:,j,:]  (+ uT)
                uc_ps = hpspool2.tile([P, c.B_LOC], F32, tag="uc")
                for j in range(c.SPP):
                    nc.tensor.matmul(
                        out=uc_ps[:],
                        lhsT=G[:, j, (h + 1) * E : (h + 2) * E],
                        rhs=bd[:, j, :],
                        start=(j == 0),
                        stop=(j == c.SPP - 1),
                    )
                uT_new = upool.tile([P, c.B_LOC], F32, tag=f"uT{h + 1}")
                nc.vector.tensor_add(out=uT_new[:], in0=uc_ps[:], in1=uT[:])
                uT = uT_new
                uT_bf_new = upool.tile([P, c.B_LOC], BF16, tag=f"uT_bf{h + 1}")
                nc.vector.tensor_copy(out=uT_bf_new[:], in_=uT[:])
                uT_bf = uT_bf_new

        # ---------- final phase: logits + vocab softmax ----------
        with (
            tc.tile_pool(name="fin", bufs=1) as fpool,
            tc.tile_pool(name="emb3c", bufs=2) as epool,
            tc.tile_pool(name="fin_ps", bufs=2, space="PSUM") as fps,
            tc.tile_pool(name="den_ps", bufs=1, space="PSUM") as dps,
            tc.tile_pool(name="out_ps", bufs=4, space="PSUM") as ops,
            tc.tile_pool(name="outsb", bufs=4) as osb,
        ):
            ones = fpool.tile([P, P], F32)
            nc.vector.memset(ones[:], 1.0)
            ones_part = fpool.tile([P, P], F32)
            nc.vector.memset(ones_part[:], 0.0)
            nc.vector.memset(ones_part[: c.LAST_VT_ROWS, :], 1.0)
            if FINAL_FP8:
                uT_fin = fpool.tile([P, c.B_LOC], F8)
                nc.vector.tensor_copy(out=uT_fin[:], in_=uT[:])
            else:
                uT_fin = uT_bf

            exp_buf = fpool.tile([P, c.NVT * c.B_LOC], F32)
            CW = c.CHUNK_VT * c.B_LOC  # psum/exp cols per chunk
            den_ps = dps.tile([P, CW], F32)
            for ch in range(c.NCH):
                vt0 = ch * c.CHUNK_VT
                nvt = min(c.CHUNK_VT, c.NVT - vt0)
                echunk = epool.tile(
                    [P, c.CHUNK_VT * P], F8 if FINAL_FP8 else BF16, tag="echunk"
                )
                nc.sync.dma_start(
                    out=echunk[:, : nvt * P],
                    in_=emb3T[:, vt0 * P : (vt0 + nvt) * P],
                )
                lg_ps = fps.tile([P, CW], F32, tag="lg")
                for m in range(nvt):
                    nc.tensor.matmul(
                        out=lg_ps[:, m * c.B_LOC : (m + 1) * c.B_LOC],
                        lhsT=echunk[:, m * P : (m + 1) * P],
                        rhs=uT_fin[:],
                        start=True,
                        stop=True,
                    )
                ecols = nvt * c.B_LOC
                nc.scalar.activation(
                    out=exp_buf[:, vt0 * c.B_LOC : vt0 * c.B_LOC + ecols],
                    in_=lg_ps[:, :ecols],
                    func=ACTF.Exp,
                )
                # denominator partials: ones^T @ exp_chunk, accumulated in psum
                exp_ch = exp_buf[:, vt0 * c.B_LOC : vt0 * c.B_LOC + ecols]
                last_has_partial = vt0 + nvt == c.NVT and c.LAST_VT_ROWS < P
                full_cols = ecols - (c.B_LOC if last_has_partial else 0)
                if full_cols > 0:
                    nc.tensor.matmul(
                        out=den_ps[:, :full_cols],
                        lhsT=ones[:],
                        rhs=exp_ch[:, :full_cols],
                        start=(ch == 0),
                        stop=False,
                        skip_group_check=True,
                    )
                if last_has_partial:
                    nc.tensor.matmul(
                        out=den_ps[:, full_cols:ecols],
                        lhsT=ones_part[:],
                        rhs=exp_ch[:, full_cols:ecols],
                        start=False,
                        stop=True,
                        skip_group_check=True,
                    )
            # denominators [1, B_LOC] then reciprocal replicated to [128,1]
            den8 = fpool.tile([P, c.B_LOC], F32)
            nc.vector.tensor_reduce(
                out=den8[:].unsqueeze(-1),
                in_=den_ps[:].rearrange("o (m b) -> o b m", b=c.B_LOC),
                axis=AX.X,
                op=ALU.add,
            )
            rec8 = fpool.tile([P, c.B_LOC], F32)
            nc.vector.reciprocal(out=rec8[:], in_=den8[:])
            # rec_rep[p] = rec8[p % B_LOC] via mask multiply + free reduce
            rec_full = fpool.tile([P, c.B_LOC], F32)
            nc.vector.tensor_tensor(
                out=rec_full[:],
                in0=bmask2_t[:],
                in1=rec8[:],
                op=ALU.mult,
            )
            rec_rep = fpool.tile([P, 1], F32)
            nc.vector.tensor_reduce(
                out=rec_rep[:], in_=rec_full[:], axis=AX.X, op=ALU.add
            )

            # transpose 16-V-tile groups, scale by recip, DMA out
            GRP = P // c.B_LOC  # V tiles per transpose group
            ngrp = -(-c.NVT // GRP)
            n_full_vt = c.V // P  # V tiles fully inside the real vocab
            out3 = out[:, : n_full_vt * P].rearrange("b (t col) -> t b col", col=P)
            for g in range(ngrp):
                t0 = g * GRP
                nt = min(GRP, c.NVT - t0)
                cols = nt * c.B_LOC
                tps = ops.tile([P, P], F32, tag="otp")
                nc.tensor.matmul(
                    out=tps[:cols, :],
                    lhsT=exp_buf[:, t0 * c.B_LOC : t0 * c.B_LOC + cols],
                    rhs=identity[:],
                    start=True,
                    stop=True,
                )
                sb = osb.tile([P, P], F32, tag="osb")
                nc.vector.tensor_scalar_mul(sb[:cols, :], tps[:cols, :], rec_rep[:cols, :])
                # rows b + B_LOC*t', t' = local V-tile; tail V tile may be partial
                full_t = min(nt, n_full_vt - t0)
                if full_t > 0:
                    nc.sync.dma_start(
                        out=out3[t0 : t0 + full_t],
                        in_=sb[: full_t * c.B_LOC, :],
                    )
                if full_t < nt:  # partial last V tile
                    nc.sync.dma_start(
                        out=out[:, n_full_vt * P : c.V],
                        in_=sb[full_t * c.B_LOC : cols, : c.V - n_full_vt * P],
                    )


# ---------------- host-side pack/unpack ----------------
def ref_numpy(story, question, emb_A):
    """Full-batch numpy reference (mirrors reference.py)."""
    K_HOP = emb_A.shape[0] - 1
    u = emb_A[0][question].sum(axis=1)
    for i in range(K_HOP):
        m = emb_A[i][story].sum(axis=2)
        cc = emb_A[i + 1][story].sum(axis=2)
        logits_att = np.einsum("bse,be->bs", m, u)
        pa = np.exp(logits_att - logits_att.max(-1, keepdims=True))
        probs = pa / pa.sum(-1, keepdims=True)
        u = np.einsum("bse,bs->be", cc, probs) + u
    logits = u @ emb_A[-1].T
    z = np.exp(logits - logits.max(-1, keepdims=True))
    return (z / z.sum(-1, keepdims=True)).astype(np.float32)


N_CORES = 8
_CACHE = {}


def _get_nc(cfg):
    key = "nc"
    if key not in _CACHE:
        import concourse.bacc as bacc

        nc = bacc.Bacc(target_bir_lowering=False)
        build_kernel(cfg, nc)
        nc.finalize()
        _CACHE[key] = nc
    return _CACHE[key]


def _pack_shared(cfg, emb_A):
    key = "shared"
    if key not in _CACHE or _CACHE[key][0] is not emb_A:
        c = cfg
        import ml_dtypes

        embs = {}
        # concat tables [V, NT*E] + zero pad row
        ecat = np.zeros((c.V + 1, c.EC), np.float32)
        ecat[: c.V] = np.concatenate([emb_A[t] for t in range(c.NT)], axis=1)
        gdt = ml_dtypes.float8_e4m3 if GATHER_FP8 else ml_dtypes.bfloat16
        embs["embcat"] = ecat.astype(gdt)
        e3T = np.zeros((E, c.VPAD), np.float32)
        e3T[:, : c.V] = emb_A[c.NT - 1].T
        fdt = ml_dtypes.float8_e4m3 if FINAL_FP8 else ml_dtypes.bfloat16
        embs["emb3T"] = e3T.astype(fdt)
        bm = np.zeros((P, c.B_LOC), np.float32)
        for b in range(c.B_LOC):
            bm[b * c.PPB : (b + 1) * c.PPB, b] = 1.0
        embs["bmask_bf"] = bm.astype(ml_dtypes.bfloat16)
        bm2 = np.zeros((P, c.B_LOC), np.float32)
        for p in range(P):
            bm2[p, p % c.B_LOC] = 1.0
        embs["bmask2"] = bm2
        _CACHE[key] = (emb_A, embs)
    return _CACHE[key][1]


def _pack_story(cfg, story_c):
    c = cfg
    story_pad = np.full((c.B_LOC, c.S_PAD, c.SENT), c.V, np.int32)
    story_pad[:, : c.S, :] = story_c
    return np.ascontiguousarray(story_pad.reshape(c.TOT_SLOTS, c.SENT))


def _pack_question(cfg, quest_c):
    """[B_LOC, SENT] -> [128, QI]: partition PPB*b + r holds tokens
    QI*r .. QI*r+QI-1 of batch b (pads -> V)."""
    c = cfg
    qp = np.full((c.B_LOC, c.PPB * c.QI), c.V, np.int32)
    qp[:, : c.SENT] = quest_c
    return np.ascontiguousarray(qp.reshape(P, c.QI))


def kernel(story, question, emb_A, _trace=False, _trace_kwargs=None):
    from concourse import bass_utils

    story = np.asarray(story)
    question = np.asarray(question)
    emb_A = np.asarray(emb_A)

    cfg = Cfg(
        B_LOC=story.shape[0] // N_CORES,
        S=story.shape[1],
        SENT=story.shape[2],
        V=emb_A.shape[1],
        K_HOP=emb_A.shape[0] - 1,
    )
    nc = _get_nc(cfg)
    shared = _pack_shared(cfg, emb_A)
    in_maps = []
    for ci in range(N_CORES):
        sl = slice(ci * cfg.B_LOC, (ci + 1) * cfg.B_LOC)
        in_maps.append(
            {
                "story_pad": _pack_story(cfg, story[sl]),
                "question": _pack_question(cfg, np.asarray(question[sl]).astype(np.int32)),
                **shared,
            }
        )
    kwargs = {}
    if _trace:
        kwargs = dict(trace=True, trace_kwargs=_trace_kwargs or {})
    res = bass_utils.run_bass_kernel_spmd(
        nc, in_maps, core_ids=list(range(N_CORES)), **kwargs
    )
    out = np.concatenate([r["out"] for r in res.results], axis=0)
    if _trace:
        return out, res
    return out


# revision 37
# speedup vs baseline: 1.2996x; 1.0169x over previous
"""MemN2N Bass kernel builder (per-core program, SPMD over 8 cores).

Per-core work (core c):
  - 8 local batches (B_LOC). story_pad [TOT_SLOTS, SENT] int32 staged so that
    slot(16b+q, j) = batch b, sentence 13q+j  (S_PAD sents/batch, SPP per part).
    Pad tokens point at table row V (a zero row appended host-side).
  - The 4 tables are concatenated host-side into embcat [V+1, 4E] bf16 so one
    gathered row covers all tables (4x fewer gather instructions than
    per-table). HW indirect DMA reads ONE offset per partition per
    instruction, so it takes SENT*SPP=650 instructions (~1.66us each, SWDGE
    fixed-cost bound).
  - Token-sum via unit-stride halving adds on DVE -> G [128, SPP, 4E] bf16,
    overlapped under the gather stream (ops kept <= 13 cols so none blocks
    the gathers for long).
  - 3 attention hops (PE transposes for G^T, scores matmul, softmax on ACT,
    DRAM-bounce repack, block-diag combine matmul) -> u3.
  - logits = u3 @ emb3.T via emb3T bf16 [E, VPAD] staged pre-transposed;
    softmax over vocab computed on-device; output [B_LOC, V] f32.
"""
import sys

sys.path.insert(0, "/opt/trn_rl_repo")

from contextlib import ExitStack

import numpy as np

import concourse.bass as bass
import concourse.mybir as mybir
import concourse.tile as tile
from concourse.masks import make_identity

F32 = mybir.dt.float32
BF16 = mybir.dt.bfloat16
F8 = mybir.dt.float8e4
I32 = mybir.dt.int32
GATHER_FP8 = False  # e4m3 embcat measured slower (DVE fp8 adds) and noisier
FINAL_FP8 = True  # e4m3 emb3T + uT: halves final-phase weight stream, faster LDWEIGHTS
AX = mybir.AxisListType
ALU = mybir.AluOpType
ACTF = mybir.ActivationFunctionType

P = 128
E = 128


class Cfg:
    def __init__(self, B_LOC=8, S=200, SENT=50, V=100000, K_HOP=3, CHUNK_VT=32):
        self.B_LOC = B_LOC
        self.S = S
        self.SENT = SENT
        self.V = V
        self.K_HOP = K_HOP
        self.NT = K_HOP + 1
        self.EC = self.NT * E  # concat row width
        self.PPB = P // B_LOC  # partitions per batch
        self.QI = -(-SENT // self.PPB)  # question tokens per partition
        # S_PAD: sentences per batch padded so B_LOC*S_PAD = 128*SPP
        self.SPP = -(-(B_LOC * S) // P)  # ceil
        self.S_PAD = self.PPB * self.SPP
        assert self.S_PAD >= S
        self.TOT_SLOTS = P * self.SPP
        # vocab padding for 128-row tiles
        self.NVT = -(-V // P)  # number of V tiles
        self.VPAD = self.NVT * P
        self.LAST_VT_ROWS = V - (self.NVT - 1) * P  # valid rows in last V tile
        # final-phase chunking: CHUNK_VT V-tiles of logits per psum/exp chunk
        self.CHUNK_VT = CHUNK_VT
        self.NCH = -(-self.NVT // CHUNK_VT)


def build_kernel(cfg: Cfg, nc: bass.Bass, dbg: bool = False):
    c = cfg
    # ---- I/O ----
    story = nc.declare_dram_parameter("story_pad", [c.TOT_SLOTS, c.SENT], I32, isOutput=False)
    # question tokens packed [128, QI]: partition PPB*b + r holds tokens
    # QI*r .. QI*r+QI-1 of batch b (pads -> row V)
    quest = nc.declare_dram_parameter("question", [P, c.QI], I32, isOutput=False)
    embcat = nc.declare_dram_parameter(
        "embcat", [c.V + 1, c.EC], F8 if GATHER_FP8 else BF16, isOutput=False
    )
    emb3T = nc.declare_dram_parameter(
        "emb3T", [E, c.VPAD], F8 if FINAL_FP8 else BF16, isOutput=False
    )
    bmask_bf = nc.declare_dram_parameter("bmask_bf", [P, c.B_LOC], BF16, isOutput=False)
    bmask2 = nc.declare_dram_parameter("bmask2", [P, c.B_LOC], F32, isOutput=False)
    out = nc.declare_dram_parameter("out", [c.B_LOC, c.V], F32, isOutput=True)

    dbgout = None
    if dbg:
        dbgout = {
            "dG": nc.declare_dram_parameter("dG", [P, c.SPP * c.EC], F32, isOutput=True),
            "duT": nc.declare_dram_parameter("duT", [P, c.B_LOC], F32, isOutput=True),
        }
    with tile.TileContext(nc) as tc:
        _body(c, nc, tc, story, quest, embcat, emb3T, bmask_bf, bmask2, out, dbgout)
    return nc


def _tree_sum(nc, scratch, src, dst, np_, ec):
    """dst[:, 0, :] = sum over the 50-token axis of src [np_, 50, ec].

    Unit-stride halving adds into a shared bf16 scratch tile [P, 48, ec].
    First level split into two ops so no single DVE op blocks the gather
    stream for long. src may be fp8; scratch/dst are bf16.
    """
    s_ = lambda a, b: scratch[:np_, a:b, :]
    # a1[i] = src[i] + src[13+i], i<13 -> cols 0:13; a2 covers 26..49 -> 13:25
    nc.vector.tensor_add(out=s_(0, 13), in0=src[:np_, 0:13, :], in1=src[:np_, 13:26, :])
    nc.vector.tensor_add(out=s_(13, 25), in0=src[:np_, 26:38, :], in1=src[:np_, 38:50, :])
    nc.vector.tensor_add(out=s_(25, 31), in0=s_(0, 6), in1=s_(6, 12))
    nc.vector.tensor_add(out=s_(31, 37), in0=s_(13, 19), in1=s_(19, 25))
    nc.vector.tensor_add(out=s_(37, 40), in0=s_(25, 28), in1=s_(28, 31))
    nc.vector.tensor_add(out=s_(40, 43), in0=s_(31, 34), in1=s_(34, 37))
    nc.vector.tensor_add(out=s_(43, 46), in0=s_(37, 40), in1=s_(40, 43))
    nc.vector.tensor_add(out=s_(46, 47), in0=s_(43, 44), in1=s_(44, 45))
    nc.vector.tensor_add(out=s_(47, 48), in0=s_(46, 47), in1=s_(45, 46))
    # + leftover a1[12] (holds src[12] + src[25])
    nc.vector.tensor_add(out=dst, in0=s_(47, 48), in1=s_(12, 13))


def _body(c: Cfg, nc, tc, story, quest, embcat, emb3T, bmask_bf, bmask2, out, dbgout=None):
    with ExitStack() as es:
        # ---------- persistent pools ----------
        cpool = es.enter_context(tc.tile_pool(name="const", bufs=1))
        gpool = es.enter_context(tc.tile_pool(name="G", bufs=1))
        upool = es.enter_context(tc.tile_pool(name="u", bufs=1))

        identity = cpool.tile([P, P], F32)
        make_identity(nc, identity[:])
        identity_bf = cpool.tile([P, P], BF16)
        nc.vector.tensor_copy(out=identity_bf[:], in_=identity[:])

        # story indices resident in SBUF: [128, SPP*SENT]
        idx_t = cpool.tile([P, c.SPP * c.SENT], I32)
        nc.sync.dma_start(
            out=idx_t[:],
            in_=story[:].rearrange("(p j) t -> p (j t)", p=P),
        )
        qidx_t = cpool.tile([P, c.QI], I32)
        nc.sync.dma_start(out=qidx_t[:], in_=quest[:])
        bmask_t = cpool.tile([P, c.B_LOC], BF16)
        nc.sync.dma_start(out=bmask_t[:], in_=bmask_bf[:])
        bmask2_t = cpool.tile([P, c.B_LOC], F32)
        nc.sync.dma_start(out=bmask2_t[:], in_=bmask2[:])

        # G concat table [128, SPP, 4E] bf16; G_t = G[:, :, t*E:(t+1)*E]
        G = gpool.tile([P, c.SPP, c.EC], BF16, tag="G", name="G")
        # G^T for m-tables [E=128, TOT_SLOTS]
        GT = [gpool.tile([P, c.TOT_SLOTS], BF16, tag=f"GT{t}", name=f"GT{t}") for t in range(c.K_HOP)]

        # ---------- gather + segment-reduce ----------
        GDT = F8 if GATHER_FP8 else BF16
        with (
            tc.tile_pool(name="gather", bufs=3 if GATHER_FP8 else 2) as gbpool,
            tc.tile_pool(name="red", bufs=1) as rpool,
            tc.tile_pool(name="tp", bufs=4, space="PSUM") as tppool,
        ):
            scratch = rpool.tile([P, 48, c.EC], BF16, tag="scr")
            for j in range(c.SPP):
                gbuf = gbpool.tile([P, c.SENT, c.EC], GDT, tag="gbuf")
                # NB: HW SWDGE reads ONE offset per partition per instruction
                # (extra offset-AP columns are ignored and the free dim is
                # filled with consecutive rows — probed); so one instruction
                # per token column, 2D out slice, exactly like the baseline.
                for s in range(c.SENT):
                    nc.gpsimd.indirect_dma_start(
                        out=gbuf[:, s, :],
                        out_offset=None,
                        in_=embcat[:],
                        in_offset=bass.IndirectOffsetOnAxis(
                            ap=idx_t[:, j * c.SENT + s : j * c.SENT + s + 1],
                            axis=0,
                        ),
                    )
                _tree_sum(nc, scratch, gbuf, G[:, j : j + 1, :], P, c.EC)
                # transpose this j's G slices into GT while later gathers run
                # (PE/DVE are otherwise idle under the gather stream)
                for t in range(c.K_HOP):
                    tp = tppool.tile([P, P], F32, tag="tp")
                    nc.tensor.matmul(
                        out=tp[:], lhsT=G[:, j, t * E : (t + 1) * E],
                        rhs=identity_bf[:], start=True, stop=True,
                    )
                    # psum col p <-> slot 13p+j: write GT[:, j::SPP]
                    nc.vector.tensor_copy(
                        out=GT[t][:].rearrange("e (p j) -> e p j", j=c.SPP)[:, :, j],
                        in_=tp[:],
                    )
            # question gather: [128, QI] indices -> per-partition partial sums
            # q4 [128, EC]; per-batch token sum finished on PE via bmask
            qbuf = gbpool.tile([P, c.SENT, c.EC], GDT, tag="gbuf")
            for s in range(c.QI):
                nc.gpsimd.indirect_dma_start(
                    out=qbuf[:, s, :],
                    out_offset=None,
                    in_=embcat[:],
                    in_offset=bass.IndirectOffsetOnAxis(
                        ap=qidx_t[:, s : s + 1], axis=0
                    ),
                )
            q4 = upool.tile([P, c.EC], BF16, tag="q4")
            assert c.QI == 4
            nc.vector.tensor_add(
                out=scratch[:, 0:1, :], in0=qbuf[:, 0:1, :], in1=qbuf[:, 1:2, :]
            )
            nc.vector.tensor_add(
                out=scratch[:, 1:2, :], in0=qbuf[:, 2:3, :], in1=qbuf[:, 3:4, :]
            )
            nc.vector.tensor_add(
                out=q4[:].unsqueeze(1), in0=scratch[:, 0:1, :], in1=scratch[:, 1:2, :]
            )

            # uT[e, b] = sum_p q4[p, e] * bmask[p, b]  (finishes the question
            # token sum across the PPB partitions of each batch, pre-transposed)
            uT = upool.tile([P, c.B_LOC], F32, tag="uT")
            uT_bf = upool.tile([P, c.B_LOC], BF16, tag="uT_bf")
            tpu = tppool.tile([P, c.B_LOC], F32, tag="tpu")
            nc.tensor.matmul(
                out=tpu[:], lhsT=q4[:, :E], rhs=bmask_t[:],
                start=True, stop=True,
            )
            nc.vector.tensor_copy(out=uT[:], in_=tpu[:])
            nc.vector.tensor_copy(out=uT_bf[:], in_=tpu[:])

        if dbgout is not None:
            # gpsimd dma casts bf16 -> f32 on the way out
            nc.gpsimd.dma_start(
                out=dbgout["dG"][:],
                in_=G[:].rearrange("p a b -> p (a b)"),
            )
            nc.sync.dma_start(out=dbgout["duT"][:], in_=uT[:])

        # ---------- K_HOP attention hops ----------
        with (
            tc.tile_pool(name="hop", bufs=2) as hpool,
            tc.tile_pool(name="hop_ps", bufs=1, space="PSUM") as hpspool,
            tc.tile_pool(name="hop_ps2", bufs=2, space="PSUM") as hpspool2,
        ):
            for h in range(c.K_HOP):
                # scores [B_LOC, TOT_SLOTS] = uT.T @ GT[h]
                sc_ps = hpspool.tile([c.B_LOC, c.TOT_SLOTS], F32, tag="sc")
                for c0 in range(0, c.TOT_SLOTS, 512):
                    c1 = min(c0 + 512, c.TOT_SLOTS)
                    nc.tensor.matmul(
                        out=sc_ps[:, c0:c1],
                        lhsT=uT_bf[:],
                        rhs=GT[h][:, c0:c1],
                        start=True,
                        stop=True,
                    )
                # move scores to SBUF, bounce via DRAM with a diagonal AP to
                # get per-batch aligned scores scal[b, s] = scores[b, S_PAD*b + s]
                sc_sb = hpool.tile([c.B_LOC, c.TOT_SLOTS], F32, tag="sc_sb")
                nc.vector.tensor_copy(out=sc_sb[:], in_=sc_ps[:])
                scd = nc.dram_tensor(f"scd{h}", [c.B_LOC * c.TOT_SLOTS], F32)
                nc.sync.dma_start(
                    out=scd[:].rearrange("(b t) -> b t", t=c.TOT_SLOTS), in_=sc_sb[:]
                )
                diag = bass.AP(
                    tensor=scd[:].tensor,
                    offset=0,
                    ap=[[c.TOT_SLOTS + c.S_PAD, c.B_LOC], [1, c.S_PAD]],
                )
                scal = hpool.tile([c.B_LOC, c.S_PAD], F32, tag="scal")
                nc.sync.dma_start(out=scal[:], in_=diag)
                # masked softmax over the S real sentences
                probs = hpool.tile([c.B_LOC, c.S_PAD], F32, tag="probs")
                nc.vector.memset(probs[:], 0.0)
                negmax = hpool.tile([c.B_LOC, 1], F32, tag="negmax")
                nc.vector.tensor_reduce(
                    out=negmax[:], in_=scal[:, : c.S], axis=AX.X, op=ALU.max, negate=True
                )
                denom = hpool.tile([c.B_LOC, 1], F32, tag="denom")
                nc.scalar.activation(
                    out=probs[:, : c.S],
                    in_=scal[:, : c.S],
                    func=ACTF.Exp,
                    bias=negmax[:],
                    scale=1.0,
                    accum_out=denom[:],
                )
                rec = hpool.tile([c.B_LOC, 1], F32, tag="rec")
                nc.vector.reciprocal(out=rec[:], in_=denom[:])
                nc.vector.tensor_scalar_mul(probs[:, : c.S], probs[:, : c.S], rec[:])
                # repack probs [B_LOC, S_PAD] -> slot layout [128, SPP] via DRAM bounce
                pd = nc.dram_tensor(f"pd{h}", [c.TOT_SLOTS], F32)
                nc.sync.dma_start(
                    out=pd[:].rearrange("(b s) -> b s", s=c.S_PAD), in_=probs[:]
                )
                pslot = hpool.tile([P, c.SPP], F32, tag="pslot")
                nc.sync.dma_start(
                    out=pslot[:], in_=pd[:].rearrange("(p j) -> p j", j=c.SPP)
                )
                pslot_bf = hpool.tile([P, c.SPP], BF16, tag="pslot_bf")
                nc.vector.tensor_copy(out=pslot_bf[:], in_=pslot[:])
                # block-diagonal probs [128, SPP, B_LOC] = pslot (bcast) * bmask (bcast)
                bd = hpool.tile([P, c.SPP, c.B_LOC], BF16, tag="bd")
                nc.vector.tensor_tensor(
                    out=bd[:],
                    in0=pslot_bf[:].unsqueeze(-1).to_broadcast([P, c.SPP, c.B_LOC]),
                    in1=bmask_t[:].unsqueeze(1).to_broadcast([P, c.SPP, c.B_LOC]),
                    op=ALU.mult,
                )
                # combine: uT_new = sum_j G[h+1][:,j,:].T @ bd[:,j,:]  (+ uT)
                uc_ps = hpspool2.tile([P, c.B_LOC], F32, tag="uc")
                for j in range(c.SPP):
                    nc.tensor.matmul(
                        out=uc_ps[:],
                        lhsT=G[:, j, (h + 1) * E : (h + 2) * E],
                        rhs=bd[:, j, :],
                        start=(j == 0),
                        stop=(j == c.SPP - 1),
                    )
                uT_new = upool.tile([P, c.B_LOC], F32, tag=f"uT{h + 1}")
                nc.vector.tensor_add(out=uT_new[:], in0=uc_ps[:], in1=uT[:])
                uT = uT_new
                uT_bf_new = upool.tile([P, c.B_LOC], BF16, tag=f"uT_bf{h + 1}")
                nc.vector.tensor_copy(out=uT_bf_new[:], in_=uT[:])
                uT_bf = uT_bf_new

        # ---------- final phase: logits + vocab softmax ----------
        with (
            tc.tile_pool(name="fin", bufs=1) as fpool,
            tc.tile_pool(name="emb3c", bufs=2) as epool,
            tc.tile_pool(name="fin_ps", bufs=2, space="PSUM") as fps,
            tc.tile_pool(name="den_ps", bufs=1, space="PSUM") as dps,
            tc.tile_pool(name="out_ps", bufs=4, space="PSUM") as ops,
            tc.tile_pool(name="outsb", bufs=4) as osb,
        ):
            ones = fpool.tile([P, P], F32)
            nc.vector.memset(ones[:], 1.0)
            ones_part = fpool.tile([P, P], F32)
            nc.vector.memset(ones_part[:], 0.0)
            nc.vector.memset(ones_part[: c.LAST_VT_ROWS, :], 1.0)
            if FINAL_FP8:
                uT_fin = fpool.tile([P, c.B_LOC], F8)
                nc.vector.tensor_copy(out=uT_fin[:], in_=uT[:])
            else:
                uT_fin = uT_bf

            exp_buf = fpool.tile([P, c.NVT * c.B_LOC], F32)
            CW = c.CHUNK_VT * c.B_LOC  # psum/exp cols per chunk
            den_ps = dps.tile([P, CW], F32)
            for ch in range(c.NCH):
                vt0 = ch * c.CHUNK_VT
                nvt = min(c.CHUNK_VT, c.NVT - vt0)
                echunk = epool.tile(
                    [P, c.CHUNK_VT * P], F8 if FINAL_FP8 else BF16, tag="echunk"
                )
                nc.sync.dma_start(
                    out=echunk[:, : nvt * P],
                    in_=emb3T[:, vt0 * P : (vt0 + nvt) * P],
                )
                lg_ps = fps.tile([P, CW], F32, tag="lg")
                for m in range(nvt):
                    nc.tensor.matmul(
                        out=lg_ps[:, m * c.B_LOC : (m + 1) * c.B_LOC],
                        lhsT=echunk[:, m * P : (m + 1) * P],
                        rhs=uT_fin[:],
                        start=True,
                        stop=True,
                    )
                ecols = nvt * c.B_LOC
                nc.scalar.activation(
                    out=exp_buf[:, vt0 * c.B_LOC : vt0 * c.B_LOC + ecols],
                    in_=lg_ps[:, :ecols],
                    func=ACTF.Exp,
                )
                # denominator partials: ones^T @ exp_chunk, accumulated in psum
                exp_ch = exp_buf[:, vt0 * c.B_LOC : vt0 * c.B_LOC + ecols]
                last_has_partial = vt0 + nvt == c.NVT and c.LAST_VT_ROWS < P
                full_cols = ecols - (c.B_LOC if last_has_partial else 0)
                if full_cols > 0:
                    nc.tensor.matmul(
                        out=den_ps[:, :full_cols],
                        lhsT=ones[:],
                        rhs=exp_ch[:, :full_cols],
                        start=(ch == 0),
                        stop=False,
                        skip_group_check=True,
                    )
                if last_has_partial:
                    nc.tensor.matmul(
                        out=den_ps[:, full_cols:ecols],
                        lhsT=ones_part[:],
                        rhs=exp_ch[:, full_cols:ecols],
                        start=False,
                        stop=True,
                        skip_group_check=True,
                    )
            # denominators [1, B_LOC] then reciprocal replicated to [128,1]
            den8 = fpool.tile([P, c.B_LOC], F32)
            nc.vector.tensor_reduce(
                out=den8[:].unsqueeze(-1),
                in_=den_ps[:].rearrange("o (m b) -> o b m", b=c.B_LOC),
                axis=AX.X,
                op=ALU.add,
            )
            rec8 = fpool.tile([P, c.B_LOC], F32)
            nc.vector.reciprocal(out=rec8[:], in_=den8[:])
            # rec_rep[p] = rec8[p % B_LOC] via mask multiply + free reduce
            rec_full = fpool.tile([P, c.B_LOC], F32)
            nc.vector.tensor_tensor(
                out=rec_full[:],
                in0=bmask2_t[:],
                in1=rec8[:],
                op=ALU.mult,
            )
            rec_rep = fpool.tile([P, 1], F32)
            nc.vector.tensor_reduce(
                out=rec_rep[:], in_=rec_full[:], axis=AX.X, op=ALU.add
            )

            # transpose 16-V-tile groups, scale by recip, DMA out
            GRP = P // c.B_LOC  # V tiles per transpose group
            ngrp = -(-c.NVT // GRP)
            n_full_vt = c.V // P  # V tiles fully inside the real vocab
            out3 = out[:, : n_full_vt * P].rearrange("b (t col) -> t b col", col=P)
            for g in range(ngrp):
                t0 = g * GRP
                nt = min(GRP, c.NVT - t0)
                cols = nt * c.B_LOC
                tps = ops.tile([P, P], F32, tag="otp")
                nc.tensor.matmul(
                    out=tps[:cols, :],
                    lhsT=exp_buf[:, t0 * c.B_LOC : t0 * c.B_LOC + cols],
                    rhs=identity[:],
                    start=True,
                    stop=True,
                )
                sb = osb.tile([P, P], F32, tag="osb")
                nc.vector.tensor_scalar_mul(sb[:cols, :], tps[:cols, :], rec_rep[:cols, :])
                # rows b + B_LOC*t', t' = local V-tile; tail V tile may be partial
                full_t = min(nt, n_full_vt - t0)
                if full_t > 0:
                    nc.sync.dma_start(
                        out=out3[t0 : t0 + full_t],
                        in_=sb[: full_t * c.B_LOC, :],
                    )
                if full_t < nt:  # partial last V tile
                    nc.sync.dma_start(
                        out=out[:, n_full_vt * P : c.V],
                        in_=sb[full_t * c.B_LOC : cols, : c.V - n_full_vt * P],
                    )


# ---------------- host-side pack/unpack ----------------
def ref_numpy(story, question, emb_A):
    """Full-batch numpy reference (mirrors reference.py)."""
    K_HOP = emb_A.shape[0] - 1
    u = emb_A[0][question].sum(axis=1)
    for i in range(K_HOP):
        m = emb_A[i][story].sum(axis=2)
        cc = emb_A[i + 1][story].sum(axis=2)
        logits_att = np.einsum("bse,be->bs", m, u)
        pa = np.exp(logits_att - logits_att.max(-1, keepdims=True))
        probs = pa / pa.sum(-1, keepdims=True)
        u = np.einsum("bse,bs->be", cc, probs) + u
    logits = u @ emb_A[-1].T
    z = np.exp(logits - logits.max(-1, keepdims=True))
    return (z / z.sum(-1, keepdims=True)).astype(np.float32)


N_CORES = 8
_CACHE = {}


def _get_nc(cfg):
    key = "nc"
    if key not in _CACHE:
        import concourse.bacc as bacc

        nc = bacc.Bacc(target_bir_lowering=False)
        build_kernel(cfg, nc)
        nc.finalize()
        _CACHE[key] = nc
    return _CACHE[key]


def _pack_shared(cfg, emb_A):
    key = "shared"
    if key not in _CACHE or _CACHE[key][0] is not emb_A:
        c = cfg
        import ml_dtypes

        embs = {}
        # concat tables [V, NT*E] + zero pad row
        ecat = np.zeros((c.V + 1, c.EC), np.float32)
        ecat[: c.V] = np.concatenate([emb_A[t] for t in range(c.NT)], axis=1)
        gdt = ml_dtypes.float8_e4m3 if GATHER_FP8 else ml_dtypes.bfloat16
        embs["embcat"] = ecat.astype(gdt)
        e3T = np.zeros((E, c.VPAD), np.float32)
        e3T[:, : c.V] = emb_A[c.NT - 1].T
        fdt = ml_dtypes.float8_e4m3 if FINAL_FP8 else ml_dtypes.bfloat16
        embs["emb3T"] = e3T.astype(fdt)
        bm = np.zeros((P, c.B_LOC), np.float32)
        for b in range(c.B_LOC):
            bm[b * c.PPB : (b + 1) * c.PPB, b] = 1.0
        embs["bmask_bf"] = bm.astype(ml_dtypes.bfloat16)
        bm2 = np.zeros((P, c.B_LOC), np.float32)
        for p in range(P):
            bm2[p, p % c.B_LOC] = 1.0
        embs["bmask2"] = bm2
        _CACHE[key] = (emb_A, embs)
    return _CACHE[key][1]


def _pack_story(cfg, story_c):
    c = cfg
    story_pad = np.full((c.B_LOC, c.S_PAD, c.SENT), c.V, np.int32)
    story_pad[:, : c.S, :] = story_c
    return np.ascontiguousarray(story_pad.reshape(c.TOT_SLOTS, c.SENT))


def _pack_question(cfg, quest_c):
    """[B_LOC, SENT] -> [128, QI]: partition PPB*b + r holds tokens
    QI*r .. QI*r+QI-1 of batch b (pads -> V)."""
    c = cfg
    qp = np.full((c.B_LOC, c.PPB * c.QI), c.V, np.int32)
    qp[:, : c.SENT] = quest_c
    return np.ascontiguousarray(qp.reshape(P, c.QI))


def kernel(story, question, emb_A, _trace=False, _trace_kwargs=None):
    from concourse import bass_utils

    story = np.asarray(story)
    question = np.asarray(question)
    emb_A = np.asarray(emb_A)

    cfg = Cfg(
        B_LOC=story.shape[0] // N_CORES,
        S=story.shape[1],
        SENT=story.shape[2],
        V=emb_A.shape[1],
        K_HOP=emb_A.shape[0] - 1,
    )
    nc = _get_nc(cfg)
    shared = _pack_shared(cfg, emb_A)
    in_maps = []
    for ci in range(N_CORES):
        sl = slice(ci * cfg.B_LOC, (ci + 1) * cfg.B_LOC)
        in_maps.append(
            {
                "story_pad": _pack_story(cfg, story[sl]),
                "question": _pack_question(cfg, np.asarray(question[sl]).astype(np.int32)),
                **shared,
            }
        )
    kwargs = {}
    if _trace:
        kwargs = dict(trace=True, trace_kwargs=_trace_kwargs or {})
    res = bass_utils.run_bass_kernel_spmd(
        nc, in_maps, core_ids=list(range(N_CORES)), **kwargs
    )
    out = np.concatenate([r["out"] for r in res.results], axis=0)
    if _trace:
        return out, res
    return out


# revision 39
# speedup vs baseline: 1.3075x; 1.0061x over previous
"""MemN2N Bass kernel builder (per-core program, SPMD over 8 cores).

Per-core work (core c):
  - 8 local batches (B_LOC). story_pad [TOT_SLOTS, SENT] int32 staged so that
    slot(16b+q, j) = batch b, sentence 13q+j  (S_PAD sents/batch, SPP per part).
    Pad tokens point at table row V (a zero row appended host-side).
  - The 4 tables are concatenated host-side into embcat [V+1, 4E] bf16 so one
    gathered row covers all tables (4x fewer gather instructions than
    per-table). HW indirect DMA reads ONE offset per partition per
    instruction, so it takes SENT*SPP=650 instructions (~1.66us each, SWDGE
    fixed-cost bound).
  - Token-sum via unit-stride halving adds on DVE -> G [128, SPP, 4E] bf16,
    overlapped under the gather stream (ops kept <= 13 cols so none blocks
    the gathers for long).
  - 3 attention hops (PE transposes for G^T, scores matmul, softmax on ACT,
    DRAM-bounce repack, block-diag combine matmul) -> u3.
  - logits = u3 @ emb3.T via emb3T bf16 [E, VPAD] staged pre-transposed;
    softmax over vocab computed on-device; output [B_LOC, V] f32.
"""
import sys

sys.path.insert(0, "/opt/trn_rl_repo")

from contextlib import ExitStack

import numpy as np

import concourse.bass as bass
import concourse.mybir as mybir
import concourse.tile as tile
from concourse.masks import make_identity

F32 = mybir.dt.float32
BF16 = mybir.dt.bfloat16
F8 = mybir.dt.float8e4
I32 = mybir.dt.int32
GATHER_FP8 = False  # e4m3 embcat measured slower (DVE fp8 adds) and noisier
FINAL_FP8 = True  # e4m3 emb3T + uT: halves final-phase weight stream, faster LDWEIGHTS
AX = mybir.AxisListType
ALU = mybir.AluOpType
ACTF = mybir.ActivationFunctionType

P = 128
E = 128


class Cfg:
    def __init__(self, B_LOC=8, S=200, SENT=50, V=100000, K_HOP=3, CHUNK_VT=64):
        self.B_LOC = B_LOC
        self.S = S
        self.SENT = SENT
        self.V = V
        self.K_HOP = K_HOP
        self.NT = K_HOP + 1
        self.EC = self.NT * E  # concat row width
        self.PPB = P // B_LOC  # partitions per batch
        self.QI = -(-SENT // self.PPB)  # question tokens per partition
        # S_PAD: sentences per batch padded so B_LOC*S_PAD = 128*SPP
        self.SPP = -(-(B_LOC * S) // P)  # ceil
        self.S_PAD = self.PPB * self.SPP
        assert self.S_PAD >= S
        self.TOT_SLOTS = P * self.SPP
        # vocab padding for 128-row tiles
        self.NVT = -(-V // P)  # number of V tiles
        self.VPAD = self.NVT * P
        self.LAST_VT_ROWS = V - (self.NVT - 1) * P  # valid rows in last V tile
        # final-phase chunking: CHUNK_VT V-tiles of logits per psum/exp chunk
        self.CHUNK_VT = CHUNK_VT
        self.NCH = -(-self.NVT // CHUNK_VT)


def build_kernel(cfg: Cfg, nc: bass.Bass, dbg: bool = False):
    c = cfg
    # ---- I/O ----
    story = nc.declare_dram_parameter("story_pad", [c.TOT_SLOTS, c.SENT], I32, isOutput=False)
    # question tokens packed [128, QI]: partition PPB*b + r holds tokens
    # QI*r .. QI*r+QI-1 of batch b (pads -> row V)
    quest = nc.declare_dram_parameter("question", [P, c.QI], I32, isOutput=False)
    embcat = nc.declare_dram_parameter(
        "embcat", [c.V + 1, c.EC], F8 if GATHER_FP8 else BF16, isOutput=False
    )
    emb3T = nc.declare_dram_parameter(
        "emb3T", [E, c.VPAD], F8 if FINAL_FP8 else BF16, isOutput=False
    )
    bmask_bf = nc.declare_dram_parameter("bmask_bf", [P, c.B_LOC], BF16, isOutput=False)
    bmask2 = nc.declare_dram_parameter("bmask2", [P, c.B_LOC], F32, isOutput=False)
    out = nc.declare_dram_parameter("out", [c.B_LOC, c.V], F32, isOutput=True)

    dbgout = None
    if dbg:
        dbgout = {
            "dG": nc.declare_dram_parameter("dG", [P, c.SPP * c.EC], F32, isOutput=True),
            "duT": nc.declare_dram_parameter("duT", [P, c.B_LOC], F32, isOutput=True),
        }
    with tile.TileContext(nc) as tc:
        _body(c, nc, tc, story, quest, embcat, emb3T, bmask_bf, bmask2, out, dbgout)
    return nc


def _tree_sum(nc, scratch, src, dst, np_, ec):
    """dst[:, 0, :] = sum over the 50-token axis of src [np_, 50, ec].

    Unit-stride halving adds into a shared bf16 scratch tile [P, 48, ec].
    First level split into two ops so no single DVE op blocks the gather
    stream for long. src may be fp8; scratch/dst are bf16.
    """
    s_ = lambda a, b: scratch[:np_, a:b, :]
    # a1[i] = src[i] + src[13+i], i<13 -> cols 0:13; a2 covers 26..49 -> 13:25
    nc.vector.tensor_add(out=s_(0, 13), in0=src[:np_, 0:13, :], in1=src[:np_, 13:26, :])
    nc.vector.tensor_add(out=s_(13, 25), in0=src[:np_, 26:38, :], in1=src[:np_, 38:50, :])
    nc.vector.tensor_add(out=s_(25, 31), in0=s_(0, 6), in1=s_(6, 12))
    nc.vector.tensor_add(out=s_(31, 37), in0=s_(13, 19), in1=s_(19, 25))
    nc.vector.tensor_add(out=s_(37, 40), in0=s_(25, 28), in1=s_(28, 31))
    nc.vector.tensor_add(out=s_(40, 43), in0=s_(31, 34), in1=s_(34, 37))
    nc.vector.tensor_add(out=s_(43, 46), in0=s_(37, 40), in1=s_(40, 43))
    nc.vector.tensor_add(out=s_(46, 47), in0=s_(43, 44), in1=s_(44, 45))
    nc.vector.tensor_add(out=s_(47, 48), in0=s_(46, 47), in1=s_(45, 46))
    # + leftover a1[12] (holds src[12] + src[25])
    nc.vector.tensor_add(out=dst, in0=s_(47, 48), in1=s_(12, 13))


def _body(c: Cfg, nc, tc, story, quest, embcat, emb3T, bmask_bf, bmask2, out, dbgout=None):
    with ExitStack() as es:
        # ---------- persistent pools ----------
        cpool = es.enter_context(tc.tile_pool(name="const", bufs=1))
        gpool = es.enter_context(tc.tile_pool(name="G", bufs=1))
        upool = es.enter_context(tc.tile_pool(name="u", bufs=1))

        identity = cpool.tile([P, P], F32)
        make_identity(nc, identity[:])
        identity_bf = cpool.tile([P, P], BF16)
        nc.vector.tensor_copy(out=identity_bf[:], in_=identity[:])

        # story indices resident in SBUF: [128, SPP*SENT]
        idx_t = cpool.tile([P, c.SPP * c.SENT], I32)
        nc.sync.dma_start(
            out=idx_t[:],
            in_=story[:].rearrange("(p j) t -> p (j t)", p=P),
        )
        qidx_t = cpool.tile([P, c.QI], I32)
        nc.sync.dma_start(out=qidx_t[:], in_=quest[:])
        bmask_t = cpool.tile([P, c.B_LOC], BF16)
        nc.sync.dma_start(out=bmask_t[:], in_=bmask_bf[:])
        bmask2_t = cpool.tile([P, c.B_LOC], F32)
        nc.sync.dma_start(out=bmask2_t[:], in_=bmask2[:])

        # G concat table [128, SPP, 4E] bf16; G_t = G[:, :, t*E:(t+1)*E]
        G = gpool.tile([P, c.SPP, c.EC], BF16, tag="G", name="G")
        # G^T for m-tables [E=128, TOT_SLOTS]
        GT = [gpool.tile([P, c.TOT_SLOTS], BF16, tag=f"GT{t}", name=f"GT{t}") for t in range(c.K_HOP)]

        # ---------- gather + segment-reduce ----------
        GDT = F8 if GATHER_FP8 else BF16
        with (
            tc.tile_pool(name="gather", bufs=3 if GATHER_FP8 else 2) as gbpool,
            tc.tile_pool(name="red", bufs=1) as rpool,
            tc.tile_pool(name="tp", bufs=4, space="PSUM") as tppool,
        ):
            scratch = rpool.tile([P, 48, c.EC], BF16, tag="scr")
            for j in range(c.SPP):
                gbuf = gbpool.tile([P, c.SENT, c.EC], GDT, tag="gbuf")
                # NB: HW SWDGE reads ONE offset per partition per instruction
                # (extra offset-AP columns are ignored and the free dim is
                # filled with consecutive rows — probed); so one instruction
                # per token column, 2D out slice, exactly like the baseline.
                for s in range(c.SENT):
                    nc.gpsimd.indirect_dma_start(
                        out=gbuf[:, s, :],
                        out_offset=None,
                        in_=embcat[:],
                        in_offset=bass.IndirectOffsetOnAxis(
                            ap=idx_t[:, j * c.SENT + s : j * c.SENT + s + 1],
                            axis=0,
                        ),
                    )
                _tree_sum(nc, scratch, gbuf, G[:, j : j + 1, :], P, c.EC)
                # transpose this j's G slices into GT while later gathers run
                # (PE/DVE are otherwise idle under the gather stream)
                for t in range(c.K_HOP):
                    tp = tppool.tile([P, P], F32, tag="tp")
                    nc.tensor.matmul(
                        out=tp[:], lhsT=G[:, j, t * E : (t + 1) * E],
                        rhs=identity_bf[:], start=True, stop=True,
                    )
                    # psum col p <-> slot 13p+j: write GT[:, j::SPP]
                    nc.vector.tensor_copy(
                        out=GT[t][:].rearrange("e (p j) -> e p j", j=c.SPP)[:, :, j],
                        in_=tp[:],
                    )
            # question gather: [128, QI] indices -> per-partition partial sums
            # q4 [128, EC]; per-batch token sum finished on PE via bmask
            qbuf = gbpool.tile([P, c.SENT, c.EC], GDT, tag="gbuf")
            for s in range(c.QI):
                nc.gpsimd.indirect_dma_start(
                    out=qbuf[:, s, :],
                    out_offset=None,
                    in_=embcat[:],
                    in_offset=bass.IndirectOffsetOnAxis(
                        ap=qidx_t[:, s : s + 1], axis=0
                    ),
                )
            q4 = upool.tile([P, c.EC], BF16, tag="q4")
            assert c.QI == 4
            nc.vector.tensor_add(
                out=scratch[:, 0:1, :], in0=qbuf[:, 0:1, :], in1=qbuf[:, 1:2, :]
            )
            nc.vector.tensor_add(
                out=scratch[:, 1:2, :], in0=qbuf[:, 2:3, :], in1=qbuf[:, 3:4, :]
            )
            nc.vector.tensor_add(
                out=q4[:].unsqueeze(1), in0=scratch[:, 0:1, :], in1=scratch[:, 1:2, :]
            )

            # uT[e, b] = sum_p q4[p, e] * bmask[p, b]  (finishes the question
            # token sum across the PPB partitions of each batch, pre-transposed)
            uT = upool.tile([P, c.B_LOC], F32, tag="uT")
            uT_bf = upool.tile([P, c.B_LOC], BF16, tag="uT_bf")
            tpu = tppool.tile([P, c.B_LOC], F32, tag="tpu")
            nc.tensor.matmul(
                out=tpu[:], lhsT=q4[:, :E], rhs=bmask_t[:],
                start=True, stop=True,
            )
            nc.vector.tensor_copy(out=uT[:], in_=tpu[:])
            nc.vector.tensor_copy(out=uT_bf[:], in_=tpu[:])

        if dbgout is not None:
            # gpsimd dma casts bf16 -> f32 on the way out
            nc.gpsimd.dma_start(
                out=dbgout["dG"][:],
                in_=G[:].rearrange("p a b -> p (a b)"),
            )
            nc.sync.dma_start(out=dbgout["duT"][:], in_=uT[:])

        # ---------- K_HOP attention hops ----------
        with (
            tc.tile_pool(name="hop", bufs=2) as hpool,
            tc.tile_pool(name="hop_ps", bufs=1, space="PSUM") as hpspool,
            tc.tile_pool(name="hop_ps2", bufs=2, space="PSUM") as hpspool2,
        ):
            for h in range(c.K_HOP):
                # scores [B_LOC, TOT_SLOTS] = uT.T @ GT[h]
                sc_ps = hpspool.tile([c.B_LOC, c.TOT_SLOTS], F32, tag="sc")
                for c0 in range(0, c.TOT_SLOTS, 512):
                    c1 = min(c0 + 512, c.TOT_SLOTS)
                    nc.tensor.matmul(
                        out=sc_ps[:, c0:c1],
                        lhsT=uT_bf[:],
                        rhs=GT[h][:, c0:c1],
                        start=True,
                        stop=True,
                    )
                # move scores to SBUF, bounce via DRAM with a diagonal AP to
                # get per-batch aligned scores scal[b, s] = scores[b, S_PAD*b + s]
                sc_sb = hpool.tile([c.B_LOC, c.TOT_SLOTS], F32, tag="sc_sb")
                nc.vector.tensor_copy(out=sc_sb[:], in_=sc_ps[:])
                scd = nc.dram_tensor(f"scd{h}", [c.B_LOC * c.TOT_SLOTS], F32)
                nc.sync.dma_start(
                    out=scd[:].rearrange("(b t) -> b t", t=c.TOT_SLOTS), in_=sc_sb[:]
                )
                diag = bass.AP(
                    tensor=scd[:].tensor,
                    offset=0,
                    ap=[[c.TOT_SLOTS + c.S_PAD, c.B_LOC], [1, c.S_PAD]],
                )
                scal = hpool.tile([c.B_LOC, c.S_PAD], F32, tag="scal")
                nc.sync.dma_start(out=scal[:], in_=diag)
                # masked softmax over the S real sentences
                probs = hpool.tile([c.B_LOC, c.S_PAD], F32, tag="probs")
                nc.vector.memset(probs[:], 0.0)
                negmax = hpool.tile([c.B_LOC, 1], F32, tag="negmax")
                nc.vector.tensor_reduce(
                    out=negmax[:], in_=scal[:, : c.S], axis=AX.X, op=ALU.max, negate=True
                )
                denom = hpool.tile([c.B_LOC, 1], F32, tag="denom")
                nc.scalar.activation(
                    out=probs[:, : c.S],
                    in_=scal[:, : c.S],
                    func=ACTF.Exp,
                    bias=negmax[:],
                    scale=1.0,
                    accum_out=denom[:],
                )
                rec = hpool.tile([c.B_LOC, 1], F32, tag="rec")
                nc.vector.reciprocal(out=rec[:], in_=denom[:])
                nc.vector.tensor_scalar_mul(probs[:, : c.S], probs[:, : c.S], rec[:])
                # repack probs [B_LOC, S_PAD] -> slot layout [128, SPP] via DRAM bounce
                pd = nc.dram_tensor(f"pd{h}", [c.TOT_SLOTS], F32)
                nc.sync.dma_start(
                    out=pd[:].rearrange("(b s) -> b s", s=c.S_PAD), in_=probs[:]
                )
                pslot = hpool.tile([P, c.SPP], F32, tag="pslot")
                nc.sync.dma_start(
                    out=pslot[:], in_=pd[:].rearrange("(p j) -> p j", j=c.SPP)
                )
                pslot_bf = hpool.tile([P, c.SPP], BF16, tag="pslot_bf")
                nc.vector.tensor_copy(out=pslot_bf[:], in_=pslot[:])
                # block-diagonal probs [128, SPP, B_LOC] = pslot (bcast) * bmask (bcast)
                bd = hpool.tile([P, c.SPP, c.B_LOC], BF16, tag="bd")
                nc.vector.tensor_tensor(
                    out=bd[:],
                    in0=pslot_bf[:].unsqueeze(-1).to_broadcast([P, c.SPP, c.B_LOC]),
                    in1=bmask_t[:].unsqueeze(1).to_broadcast([P, c.SPP, c.B_LOC]),
                    op=ALU.mult,
                )
                # combine: uT_new = sum_j G[h+1][:,j,:].T @ bd[:,j,:]  (+ uT)
                uc_ps = hpspool2.tile([P, c.B_LOC], F32, tag="uc")
                for j in range(c.SPP):
                    nc.tensor.matmul(
                        out=uc_ps[:],
                        lhsT=G[:, j, (h + 1) * E : (h + 2) * E],
                        rhs=bd[:, j, :],
                        start=(j == 0),
                        stop=(j == c.SPP - 1),
                    )
                uT_new = upool.tile([P, c.B_LOC], F32, tag=f"uT{h + 1}")
                nc.vector.tensor_add(out=uT_new[:], in0=uc_ps[:], in1=uT[:])
                uT = uT_new
                uT_bf_new = upool.tile([P, c.B_LOC], BF16, tag=f"uT_bf{h + 1}")
                nc.vector.tensor_copy(out=uT_bf_new[:], in_=uT[:])
                uT_bf = uT_bf_new

        # ---------- final phase: logits + vocab softmax ----------
        with (
            tc.tile_pool(name="fin", bufs=1) as fpool,
            tc.tile_pool(name="emb3c", bufs=2) as epool,
            tc.tile_pool(name="fin_ps", bufs=3, space="PSUM") as fps,
            tc.tile_pool(name="den_ps", bufs=1, space="PSUM") as dps,
            tc.tile_pool(name="out_ps", bufs=4, space="PSUM") as ops,
            tc.tile_pool(name="outsb", bufs=4) as osb,
        ):
            ones = fpool.tile([P, P], F32)
            nc.vector.memset(ones[:], 1.0)
            ones_part = fpool.tile([P, P], F32)
            nc.vector.memset(ones_part[:], 0.0)
            nc.vector.memset(ones_part[: c.LAST_VT_ROWS, :], 1.0)
            if FINAL_FP8:
                uT_fin = fpool.tile([P, c.B_LOC], F8)
                nc.vector.tensor_copy(out=uT_fin[:], in_=uT[:])
            else:
                uT_fin = uT_bf

            exp_buf = fpool.tile([P, c.NVT * c.B_LOC], F32)
            CW = c.CHUNK_VT * c.B_LOC  # psum/exp cols per chunk
            den_ps = dps.tile([P, CW], F32)
            for ch in range(c.NCH):
                vt0 = ch * c.CHUNK_VT
                nvt = min(c.CHUNK_VT, c.NVT - vt0)
                echunk = epool.tile(
                    [P, c.CHUNK_VT * P], F8 if FINAL_FP8 else BF16, tag="echunk"
                )
                nc.sync.dma_start(
                    out=echunk[:, : nvt * P],
                    in_=emb3T[:, vt0 * P : (vt0 + nvt) * P],
                )
                lg_ps = fps.tile([P, CW], F32, tag="lg")
                for m in range(nvt):
                    nc.tensor.matmul(
                        out=lg_ps[:, m * c.B_LOC : (m + 1) * c.B_LOC],
                        lhsT=echunk[:, m * P : (m + 1) * P],
                        rhs=uT_fin[:],
                        start=True,
                        stop=True,
                    )
                ecols = nvt * c.B_LOC
                nc.scalar.activation(
                    out=exp_buf[:, vt0 * c.B_LOC : vt0 * c.B_LOC + ecols],
                    in_=lg_ps[:, :ecols],
                    func=ACTF.Exp,
                )
                # denominator partials: ones^T @ exp_chunk, accumulated in psum
                exp_ch = exp_buf[:, vt0 * c.B_LOC : vt0 * c.B_LOC + ecols]
                last_has_partial = vt0 + nvt == c.NVT and c.LAST_VT_ROWS < P
                full_cols = ecols - (c.B_LOC if last_has_partial else 0)
                if full_cols > 0:
                    nc.tensor.matmul(
                        out=den_ps[:, :full_cols],
                        lhsT=ones[:],
                        rhs=exp_ch[:, :full_cols],
                        start=(ch == 0),
                        stop=False,
                        skip_group_check=True,
                    )
                if last_has_partial:
                    nc.tensor.matmul(
                        out=den_ps[:, full_cols:ecols],
                        lhsT=ones_part[:],
                        rhs=exp_ch[:, full_cols:ecols],
                        start=False,
                        stop=True,
                        skip_group_check=True,
                    )
            # denominators [1, B_LOC] then reciprocal replicated to [128,1]
            den8 = fpool.tile([P, c.B_LOC], F32)
            nc.vector.tensor_reduce(
                out=den8[:].unsqueeze(-1),
                in_=den_ps[:].rearrange("o (m b) -> o b m", b=c.B_LOC),
                axis=AX.X,
                op=ALU.add,
            )
            rec8 = fpool.tile([P, c.B_LOC], F32)
            nc.vector.reciprocal(out=rec8[:], in_=den8[:])
            # rec_rep[p] = rec8[p % B_LOC] via mask multiply + free reduce
            rec_full = fpool.tile([P, c.B_LOC], F32)
            nc.vector.tensor_tensor(
                out=rec_full[:],
                in0=bmask2_t[:],
                in1=rec8[:],
                op=ALU.mult,
            )
            rec_rep = fpool.tile([P, 1], F32)
            nc.vector.tensor_reduce(
                out=rec_rep[:], in_=rec_full[:], axis=AX.X, op=ALU.add
            )

            # transpose 16-V-tile groups, scale by recip, DMA out
            GRP = P // c.B_LOC  # V tiles per transpose group
            ngrp = -(-c.NVT // GRP)
            n_full_vt = c.V // P  # V tiles fully inside the real vocab
            out3 = out[:, : n_full_vt * P].rearrange("b (t col) -> t b col", col=P)
            for g in range(ngrp):
                t0 = g * GRP
                nt = min(GRP, c.NVT - t0)
                cols = nt * c.B_LOC
                tps = ops.tile([P, P], F32, tag="otp")
                nc.tensor.matmul(
                    out=tps[:cols, :],
                    lhsT=exp_buf[:, t0 * c.B_LOC : t0 * c.B_LOC + cols],
                    rhs=identity[:],
                    start=True,
                    stop=True,
                )
                sb = osb.tile([P, P], F32, tag="osb")
                nc.vector.tensor_scalar_mul(sb[:cols, :], tps[:cols, :], rec_rep[:cols, :])
                # rows b + B_LOC*t', t' = local V-tile; tail V tile may be partial
                full_t = min(nt, n_full_vt - t0)
                if full_t > 0:
                    nc.sync.dma_start(
                        out=out3[t0 : t0 + full_t],
                        in_=sb[: full_t * c.B_LOC, :],
                    )
                if full_t < nt:  # partial last V tile
                    nc.sync.dma_start(
                        out=out[:, n_full_vt * P : c.V],
                        in_=sb[full_t * c.B_LOC : cols, : c.V - n_full_vt * P],
                    )


# ---------------- host-side pack/unpack ----------------
def ref_numpy(story, question, emb_A):
    """Full-batch numpy reference (mirrors reference.py)."""
    K_HOP = emb_A.shape[0] - 1
    u = emb_A[0][question].sum(axis=1)
    for i in range(K_HOP):
        m = emb_A[i][story].sum(axis=2)
        cc = emb_A[i + 1][story].sum(axis=2)
        logits_att = np.einsum("bse,be->bs", m, u)
        pa = np.exp(logits_att - logits_att.max(-1, keepdims=True))
        probs = pa / pa.sum(-1, keepdims=True)
        u = np.einsum("bse,bs->be", cc, probs) + u
    logits = u @ emb_A[-1].T
    z = np.exp(logits - logits.max(-1, keepdims=True))
    return (z / z.sum(-1, keepdims=True)).astype(np.float32)


N_CORES = 8
_CACHE = {}


def _get_nc(cfg):
    key = "nc"
    if key not in _CACHE:
        import concourse.bacc as bacc

        nc = bacc.Bacc(target_bir_lowering=False)
        build_kernel(cfg, nc)
        nc.finalize()
        _CACHE[key] = nc
    return _CACHE[key]


def _pack_shared(cfg, emb_A):
    key = "shared"
    if key not in _CACHE or _CACHE[key][0] is not emb_A:
        c = cfg
        import ml_dtypes

        embs = {}
        # concat tables [V, NT*E] + zero pad row
        ecat = np.zeros((c.V + 1, c.EC), np.float32)
        ecat[: c.V] = np.concatenate([emb_A[t] for t in range(c.NT)], axis=1)
        gdt = ml_dtypes.float8_e4m3 if GATHER_FP8 else ml_dtypes.bfloat16
        embs["embcat"] = ecat.astype(gdt)
        e3T = np.zeros((E, c.VPAD), np.float32)
        e3T[:, : c.V] = emb_A[c.NT - 1].T
        fdt = ml_dtypes.float8_e4m3 if FINAL_FP8 else ml_dtypes.bfloat16
        embs["emb3T"] = e3T.astype(fdt)
        bm = np.zeros((P, c.B_LOC), np.float32)
        for b in range(c.B_LOC):
            bm[b * c.PPB : (b + 1) * c.PPB, b] = 1.0
        embs["bmask_bf"] = bm.astype(ml_dtypes.bfloat16)
        bm2 = np.zeros((P, c.B_LOC), np.float32)
        for p in range(P):
            bm2[p, p % c.B_LOC] = 1.0
        embs["bmask2"] = bm2
        _CACHE[key] = (emb_A, embs)
    return _CACHE[key][1]


def _pack_story(cfg, story_c):
    c = cfg
    story_pad = np.full((c.B_LOC, c.S_PAD, c.SENT), c.V, np.int32)
    story_pad[:, : c.S, :] = story_c
    return np.ascontiguousarray(story_pad.reshape(c.TOT_SLOTS, c.SENT))


def _pack_question(cfg, quest_c):
    """[B_LOC, SENT] -> [128, QI]: partition PPB*b + r holds tokens
    QI*r .. QI*r+QI-1 of batch b (pads -> V)."""
    c = cfg
    qp = np.full((c.B_LOC, c.PPB * c.QI), c.V, np.int32)
    qp[:, : c.SENT] = quest_c
    return np.ascontiguousarray(qp.reshape(P, c.QI))


def kernel(story, question, emb_A, _trace=False, _trace_kwargs=None):
    from concourse import bass_utils

    story = np.asarray(story)
    question = np.asarray(question)
    emb_A = np.asarray(emb_A)

    cfg = Cfg(
        B_LOC=story.shape[0] // N_CORES,
        S=story.shape[1],
        SENT=story.shape[2],
        V=emb_A.shape[1],
        K_HOP=emb_A.shape[0] - 1,
    )
    nc = _get_nc(cfg)
    shared = _pack_shared(cfg, emb_A)
    in_maps = []
    for ci in range(N_CORES):
        sl = slice(ci * cfg.B_LOC, (ci + 1) * cfg.B_LOC)
        in_maps.append(
            {
                "story_pad": _pack_story(cfg, story[sl]),
                "question": _pack_question(cfg, np.asarray(question[sl]).astype(np.int32)),
                **shared,
            }
        )
    kwargs = {}
    if _trace:
        kwargs = dict(trace=True, trace_kwargs=_trace_kwargs or {})
    res = bass_utils.run_bass_kernel_spmd(
        nc, in_maps, core_ids=list(range(N_CORES)), **kwargs
    )
    out = np.concatenate([r["out"] for r in res.results], axis=0)
    if _trace:
        return out, res
    return out
